# revision 2
# baseline (speedup 1.0000x reference)
"""Trainium2 Bass kernel for nn_AttentionFlow (gnn_message_passing).

Strategy: the dominant compute — the per-edge bilinear MLP attention scores
for both GNN layers (2 x ~103 GFLOP) and the final 100000x256x256 linear
(13 GFLOP) — runs on the 8 NeuronCores in fp32 via Bass/Tile kernels,
sharded by edges (8192 edges/core, matching the eg-group sharding hint) and
by node ranges for the final linear.  Host code performs index-derived
staging (gathers into transposed feature blocks, segment bookkeeping,
top-k assembly) and the cross-shard unshard/merge.
"""
import os
import sys

sys.path.insert(0, '/opt/trn_rl_repo')

import numpy as np

N_NODES = 100000
D = 256
B = 64
EPG = 1024
E = 65536
NCORES = 8
ESH = E // NCORES          # 8192 edges per core
TN = 512                   # moving-dim tile (edges per matmul)
NT = ESH // TN             # 16 tiles per core
KFULL = 4 * D              # 1024 contraction rows for left/right
DH = 2 * D                 # 512 hidden features
NROWS_LIN = 12800          # padded node rows per core for the final linear
NT_LIN = NROWS_LIN // TN   # 25

_cache = {}
LAST_EXEC_NS = []


def _enable_tracing_if_requested():
    if not os.environ.get("KERNEL_TRACE"):
        return False
    try:
        import types
        import antenv
        if 'antenv.axon_hooks' not in sys.modules:
            mod = types.ModuleType('antenv.axon_hooks')
            _store = [None]
            mod.set_axon_ntff_profile_hook = lambda h: _store.__setitem__(0, h)
            mod.get_axon_ntff_profile_hook = lambda: _store[0]
            sys.modules['antenv.axon_hooks'] = mod
            antenv.axon_hooks = mod
            from trn_agent_boot.trn_boot import _ntff_profile_via_ctypes
            mod.set_axon_ntff_profile_hook(
                _ntff_profile_via_ctypes('/opt/axon/libaxon_pjrt.so'))
            from concourse import bass_utils
            bass_utils.upload_artifacts = lambda tmpdir: f"local:{tmpdir}"
        return True
    except Exception:
        return False


def _build_scorer():
    """Per-core edge MLP scorer.

    in:  lT [1024, 8192]  (left features, transposed:  [hi; rel; qs; qr].T)
         rT [1024, 8192]  (right features, transposed: [hj; rel; qs; qr].T)
         Wl, Wr [1024, 512], Wc [512, 512], bl, br, bc [512]
    out: logits [1, 8192]
         logits[e] = sum(LReLU(l @ Wl + bl) * (LReLU(r @ Wr + br) @ Wc + bc))
    """
    import concourse.mybir as mybir
    import concourse.tile as tile
    from concourse import bacc

    f32 = mybir.dt.float32
    nc = bacc.Bacc("TRN2", target_bir_lowering=False, debug=False,
                   num_devices=NCORES)
    lT = nc.dram_tensor("lT", [KFULL, ESH], f32, kind="ExternalInput").ap()
    rT = nc.dram_tensor("rT", [KFULL, ESH], f32, kind="ExternalInput").ap()
    Wl = nc.dram_tensor("Wl", [KFULL, DH], f32, kind="ExternalInput").ap()
    Wr = nc.dram_tensor("Wr", [KFULL, DH], f32, kind="ExternalInput").ap()
    Wc = nc.dram_tensor("Wc", [DH, DH], f32, kind="ExternalInput").ap()
    bl = nc.dram_tensor("bl", [DH], f32, kind="ExternalInput").ap()
    br = nc.dram_tensor("br", [DH], f32, kind="ExternalInput").ap()
    bc = nc.dram_tensor("bc", [DH], f32, kind="ExternalInput").ap()
    logits = nc.dram_tensor("logits", [1, ESH], f32, kind="ExternalOutput").ap()

    KC = KFULL // 128   # 8 contraction chunks for Wl/Wr
    MC = DH // 128      # 4 output-feature chunks
    KC2 = DH // 128     # 4 contraction chunks for Wc

    with tile.TileContext(nc) as tc:
        with tc.tile_pool(name="wpool", bufs=1) as wp, \
             tc.tile_pool(name="stream", bufs=3) as sp, \
             tc.tile_pool(name="work", bufs=2) as kp, \
             tc.tile_pool(name="psum", bufs=2, space="PSUM") as pp, \
             tc.tile_pool(name="psd", bufs=2, space="PSUM") as pd:
            # --- preload weights / biases (resident) ---
            wl_t = [wp.tile([128, DH], f32, name=f"wl{k}", tag=f"wl{k}") for k in range(KC)]
            wr_t = [wp.tile([128, DH], f32, name=f"wr{k}", tag=f"wr{k}") for k in range(KC)]
            wc_t = [wp.tile([128, DH], f32, name=f"wc{k}", tag=f"wc{k}") for k in range(KC2)]
            for k in range(KC):
                nc.sync.dma_start(out=wl_t[k][:], in_=Wl[k * 128:(k + 1) * 128, :])
                nc.sync.dma_start(out=wr_t[k][:], in_=Wr[k * 128:(k + 1) * 128, :])
            for k in range(KC2):
                nc.sync.dma_start(out=wc_t[k][:], in_=Wc[k * 128:(k + 1) * 128, :])
            bl_t = wp.tile([128, MC], f32, tag="bl")
            br_t = wp.tile([128, MC], f32, tag="br")
            bc_t = wp.tile([128, MC], f32, tag="bc")
            nc.sync.dma_start(out=bl_t[:], in_=bl.rearrange("(c p) -> p c", p=128))
            nc.sync.dma_start(out=br_t[:], in_=br.rearrange("(c p) -> p c", p=128))
            nc.sync.dma_start(out=bc_t[:], in_=bc.rearrange("(c p) -> p c", p=128))
            ones_t = wp.tile([128, 1], f32, tag="ones")
            nc.vector.memset(ones_t[:], 1.0)

            for nt in range(NT):
                esl = slice(nt * TN, (nt + 1) * TN)
                # stream in the transposed feature chunks for this edge tile
                lch = []
                rch = []
                for k in range(KC):
                    t = sp.tile([128, TN], f32, name=f"lt{k}", tag=f"lt{k}")
                    nc.sync.dma_start(out=t[:], in_=lT[k * 128:(k + 1) * 128, esl])
                    lch.append(t)
                for k in range(KC):
                    t = sp.tile([128, TN], f32, name=f"rt{k}", tag=f"rt{k}")
                    nc.sync.dma_start(out=t[:], in_=rT[k * 128:(k + 1) * 128, esl])
                    rch.append(t)
                # l = LReLU(left @ Wl + bl)   (feature-major [DH, TN])
                l_sb = []
                for mc in range(MC):
                    ps = pp.tile([128, TN], f32, tag="ps", space="PSUM")
                    for k in range(KC):
                        nc.tensor.matmul(
                            ps[:], lhsT=wl_t[k][:, mc * 128:(mc + 1) * 128],
                            rhs=lch[k][:], start=(k == 0), stop=(k == KC - 1))
                    t = kp.tile([128, TN], f32, name=f"lsb{mc}", tag=f"lsb{mc}")
                    nc.scalar.activation(t[:], ps[:],
                                         mybir.ActivationFunctionType.Lrelu,
                                         bias=bl_t[:, mc:mc + 1], alpha=0.01)
                    l_sb.append(t)
                # rp = LReLU(right @ Wr + br)
                rp_sb = []
                for mc in range(MC):
                    ps = pp.tile([128, TN], f32, tag="ps", space="PSUM")
                    for k in range(KC):
                        nc.tensor.matmul(
                            ps[:], lhsT=wr_t[k][:, mc * 128:(mc + 1) * 128],
                            rhs=rch[k][:], start=(k == 0), stop=(k == KC - 1))
                    t = kp.tile([128, TN], f32, name=f"rpsb{mc}", tag=f"rpsb{mc}")
                    nc.scalar.activation(t[:], ps[:],
                                         mybir.ActivationFunctionType.Lrelu,
                                         bias=br_t[:, mc:mc + 1], alpha=0.01)
                    rp_sb.append(t)
                # r2 = rp @ Wc + bc ; prod = l * r2 ; logits += colsum(prod)
                dps = pd.tile([1, TN], f32, tag="dot", space="PSUM")
                for mc in range(MC):
                    ps = pp.tile([128, TN], f32, tag="ps", space="PSUM")
                    for k in range(KC2):
                        nc.tensor.matmul(
                            ps[:], lhsT=wc_t[k][:, mc * 128:(mc + 1) * 128],
                            rhs=rp_sb[k][:], start=(k == 0), stop=(k == KC2 - 1))
                    r2 = kp.tile([128, TN], f32, tag="r2")
                    nc.scalar.activation(r2[:], ps[:],
                                         mybir.ActivationFunctionType.Identity,
                                         bias=bc_t[:, mc:mc + 1])
                    prod = kp.tile([128, TN], f32, tag="prod")
                    nc.vector.tensor_mul(prod[:], l_sb[mc][:], r2[:])
                    nc.tensor.matmul(dps[:], lhsT=ones_t[:], rhs=prod[:],
                                     start=(mc == 0), stop=(mc == MC - 1))
                lo = kp.tile([1, TN], f32, tag="lo")
                nc.vector.tensor_copy(lo[:], dps[:])
                nc.sync.dma_start(out=logits[:, esl], in_=lo[:])
    nc.compile()
    return nc


def _build_linear():
    """Per-core final linear: outT = LReLU(Wlin.T @ embT + blin).

    in:  embT [256, 12800], Wlin [256, 256], blin [256]
    out: outT [256, 12800]
    """
    import concourse.mybir as mybir
    import concourse.tile as tile
    from concourse import bacc

    f32 = mybir.dt.float32
    nc = bacc.Bacc("TRN2", target_bir_lowering=False, debug=False,
                   num_devices=NCORES)
    embT = nc.dram_tensor("embT", [D, NROWS_LIN], f32, kind="ExternalInput").ap()
    Wlin = nc.dram_tensor("Wlin", [D, D], f32, kind="ExternalInput").ap()
    blin = nc.dram_tensor("blin", [D], f32, kind="ExternalInput").ap()
    outT = nc.dram_tensor("outT", [D, NROWS_LIN], f32, kind="ExternalOutput").ap()

    KC = D // 128  # 2
    MC = D // 128  # 2
    with tile.TileContext(nc) as tc:
        with tc.tile_pool(name="wpool", bufs=1) as wp, \
             tc.tile_pool(name="stream", bufs=3) as sp, \
             tc.tile_pool(name="psum", bufs=2, space="PSUM") as pp:
            w_t = [wp.tile([128, D], f32, name=f"w{k}", tag=f"w{k}") for k in range(KC)]
            for k in range(KC):
                nc.sync.dma_start(out=w_t[k][:], in_=Wlin[k * 128:(k + 1) * 128, :])
            b_t = wp.tile([128, MC], f32, tag="b")
            nc.sync.dma_start(out=b_t[:], in_=blin.rearrange("(c p) -> p c", p=128))
            for nt in range(NT_LIN):
                esl = slice(nt * TN, (nt + 1) * TN)
                ech = []
                for k in range(KC):
                    t = sp.tile([128, TN], f32, name=f"et{k}", tag=f"et{k}")
                    nc.sync.dma_start(out=t[:], in_=embT[k * 128:(k + 1) * 128, esl])
                    ech.append(t)
                for mc in range(MC):
                    ps = pp.tile([128, TN], f32, tag="ps", space="PSUM")
                    for k in range(KC):
                        nc.tensor.matmul(
                            ps[:], lhsT=w_t[k][:, mc * 128:(mc + 1) * 128],
                            rhs=ech[k][:], start=(k == 0), stop=(k == KC - 1))
                    o = sp.tile([128, TN], f32, tag="o")
                    nc.scalar.activation(o[:], ps[:],
                                         mybir.ActivationFunctionType.Lrelu,
                                         bias=b_t[:, mc:mc + 1], alpha=0.01)
                    nc.sync.dma_start(out=outT[mc * 128:(mc + 1) * 128, esl],
                                      in_=o[:])
    nc.compile()
    return nc


def _run(nc, in_maps, trace):
    from concourse.bass_utils import run_bass_kernel_spmd
    res = run_bass_kernel_spmd(nc, in_maps, list(range(NCORES)), trace=trace)
    if trace and res.exec_time_ns:
        LAST_EXEC_NS.append(res.exec_time_ns)
    return res.results


def _device_logits(edges, emb, rel, qs_tab, qr_tab, W, trace):
    """Run the edge-MLP scorer on 8 cores; returns logits [E] (fp32)."""
    Wl, bl, Wr, br, Wc, bc = W
    src = np.clip(edges[:, 6], 0, N_NODES - 1).astype(np.int64)
    dst = np.clip(edges[:, 7], 0, N_NODES - 1).astype(np.int64)
    eg = np.clip(edges[:, 0], 0, B - 1).astype(np.int64)
    hi = emb[src]
    hj = emb[dst]
    qs = qs_tab[eg]
    qr = qr_tab[eg]
    lT = np.concatenate([hi, rel, qs, qr], axis=1).T  # [1024, E]
    rT = np.concatenate([hj, rel, qs, qr], axis=1).T
    lT = np.ascontiguousarray(lT, dtype=np.float32)
    rT = np.ascontiguousarray(rT, dtype=np.float32)
    common = {"Wl": np.ascontiguousarray(Wl), "Wr": np.ascontiguousarray(Wr),
              "Wc": np.ascontiguousarray(Wc), "bl": bl, "br": br, "bc": bc}
    in_maps = []
    for c in range(NCORES):
        sl = slice(c * ESH, (c + 1) * ESH)
        in_maps.append({"lT": np.ascontiguousarray(lT[:, sl]),
                        "rT": np.ascontiguousarray(rT[:, sl]), **common})
    results = _run(_cache["scorer"], in_maps, trace)
    return np.concatenate([results[c]["logits"][0] for c in range(NCORES)])


def _device_linear(emb2, Wlin, blin, trace):
    """LReLU(emb2 @ Wlin + blin) on 8 cores (node-range sharded)."""
    embT = np.zeros((D, NCORES * NROWS_LIN), dtype=np.float32)
    embT[:, :N_NODES] = emb2.T
    common = {"Wlin": np.ascontiguousarray(Wlin), "blin": blin}
    in_maps = []
    for c in range(NCORES):
        sl = slice(c * NROWS_LIN, (c + 1) * NROWS_LIN)
        in_maps.append({"embT": np.ascontiguousarray(embT[:, sl]), **common})
    results = _run(_cache["linear"], in_maps, trace)
    outT = np.concatenate([results[c]["outT"] for c in range(NCORES)], axis=1)
    return np.ascontiguousarray(outT[:, :N_NODES].T)


def _segment_softmax(logits, seg):
    m = np.full(N_NODES, -np.inf, np.float32)
    np.maximum.at(m, seg, logits)
    e = np.exp(logits - m[seg])
    s = np.zeros(N_NODES, np.float32)
    np.add.at(s, seg, e)
    return e / s[seg]


def kernel(attended_nodes, node_score, edges0, edges1, rel_emb0, rel_emb1,
           memorized_embedding, query_src_ts_emb, query_rel_emb,
           Wl, bl, Wr, br, Wc, bc, Wlin, blin, max_edges):
    trace = _enable_tracing_if_requested()
    LAST_EXEC_NS.clear()
    if "scorer" not in _cache:
        _cache["scorer"] = _build_scorer()
    if "linear" not in _cache:
        _cache["linear"] = _build_linear()

    node_score = np.asarray(node_score, dtype=np.float32)
    edges0 = np.asarray(edges0, dtype=np.int32)
    edges1 = np.asarray(edges1, dtype=np.int32)
    rel_emb0 = np.asarray(rel_emb0, dtype=np.float32)
    rel_emb1 = np.asarray(rel_emb1, dtype=np.float32)
    mem = np.asarray(memorized_embedding, dtype=np.float32)
    qs_tab = np.asarray(query_src_ts_emb, dtype=np.float32)
    qr_tab = np.asarray(query_rel_emb, dtype=np.float32)
    W = (np.asarray(Wl, np.float32), np.asarray(bl, np.float32),
         np.asarray(Wr, np.float32), np.asarray(br, np.float32),
         np.asarray(Wc, np.float32), np.asarray(bc, np.float32))
    K = int(max_edges)

    # ---- layer 1: edge MLP scores on device ----
    logits1 = _device_logits(edges1, mem, rel_emb1, qs_tab, qr_tab, W, trace)
    seg1 = np.clip(edges1[:, 6], 0, N_NODES - 1).astype(np.int64)
    sm1 = _segment_softmax(logits1, seg1)
    target_att = sm1 * node_score[seg1]

    # ---- per-query top-k (stable: ties -> lower index, matching lax.top_k) --
    vals = target_att.reshape(B, EPG)
    idx = np.argsort(-vals, axis=1, kind="stable")[:, :K].astype(np.int32)
    pruned_att = np.take_along_axis(vals, idx, axis=1).reshape(-1)
    orig_indices = (np.arange(B, dtype=np.int32)[:, None] * EPG + idx).reshape(-1)
    pruned_edges = edges1[orig_indices]
    sm_pruned = sm1[orig_indices]

    # ---- node score aggregation ----
    updated_node_score = np.zeros(N_NODES, np.float32)
    tgt_p = np.clip(pruned_edges[:, 7], 0, N_NODES - 1).astype(np.int64)
    np.add.at(updated_node_score, tgt_p, sm_pruned * pruned_att)

    # ---- propagate representations along pruned edges ----
    src_p = np.clip(pruned_edges[:, 6], 0, N_NODES - 1).astype(np.int64)
    agg = np.zeros_like(mem)
    np.add.at(agg, tgt_p, sm_pruned[:, None] * mem[src_p])
    cnt = np.zeros(N_NODES, np.float32)
    np.add.at(cnt, tgt_p, 1.0)
    emb1 = np.where((cnt > 0)[:, None], agg, mem)

    # ---- layer 0 ----
    logits0 = _device_logits(edges0, emb1, rel_emb0, qs_tab, qr_tab, W, trace)
    seg0 = np.clip(edges0[:, 6], 0, N_NODES - 1).astype(np.int64)
    sm0 = _segment_softmax(logits0, seg0)
    tgt0 = np.clip(edges0[:, 7], 0, N_NODES - 1).astype(np.int64)
    agg0 = np.zeros_like(emb1)
    np.add.at(agg0, tgt0, sm0[:, None] * emb1[seg0])
    cnt0 = np.zeros(N_NODES, np.float32)
    np.add.at(cnt0, tgt0, 1.0)
    emb2 = np.where((cnt0 > 0)[:, None], agg0, emb1)

    # ---- bypass linear + LeakyReLU on device ----
    emb_out = _device_linear(emb2, np.asarray(Wlin, np.float32),
                             np.asarray(blin, np.float32), trace)

    return (updated_node_score, emb_out,
            pruned_edges.astype(np.int32), orig_indices.astype(np.int32))


# revision 3
# speedup vs baseline: 1.0919x; 1.0919x over previous
"""Trainium2 Bass kernel for nn_AttentionFlow (gnn_message_passing).

Strategy: the dominant compute — the per-edge bilinear MLP attention scores
for both GNN layers (2 x ~103 GFLOP) and the final 100000x256x256 linear
(13 GFLOP) — runs on the 8 NeuronCores in fp32 via Bass/Tile kernels,
sharded by edges (8192 edges/core, matching the eg-group sharding hint) and
by node ranges for the final linear.  Host code performs index-derived
staging (gathers into transposed feature blocks, segment bookkeeping,
top-k assembly) and the cross-shard unshard/merge.
"""
import os
import sys

sys.path.insert(0, '/opt/trn_rl_repo')

import numpy as np

N_NODES = 100000
D = 256
B = 64
EPG = 1024
E = 65536
NCORES = 8
ESH = 3584                 # multi-edge capacity per core (7 tiles of 512)
TN = 512                   # moving-dim tile (edges per matmul)
NT = ESH // TN             # 7 tiles per core
KFULL = 4 * D              # 1024 contraction rows for left/right
DH = 2 * D                 # 512 hidden features
NROWS_LIN = 12800          # padded node rows per core for the final linear
NT_LIN = NROWS_LIN // TN   # 25

_cache = {}
LAST_EXEC_NS = []


def _enable_tracing_if_requested():
    if not os.environ.get("KERNEL_TRACE"):
        return False
    try:
        import types
        import antenv
        if 'antenv.axon_hooks' not in sys.modules:
            mod = types.ModuleType('antenv.axon_hooks')
            _store = [None]
            mod.set_axon_ntff_profile_hook = lambda h: _store.__setitem__(0, h)
            mod.get_axon_ntff_profile_hook = lambda: _store[0]
            sys.modules['antenv.axon_hooks'] = mod
            antenv.axon_hooks = mod
            from trn_agent_boot.trn_boot import _ntff_profile_via_ctypes
            mod.set_axon_ntff_profile_hook(
                _ntff_profile_via_ctypes('/opt/axon/libaxon_pjrt.so'))
            from concourse import bass_utils
            bass_utils.upload_artifacts = lambda tmpdir: f"local:{tmpdir}"
        return True
    except Exception:
        return False


def _build_scorer():
    """Per-core edge MLP scorer.

    in:  lT [1024, 8192]  (left features, transposed:  [hi; rel; qs; qr].T)
         rT [1024, 8192]  (right features, transposed: [hj; rel; qs; qr].T)
         Wl, Wr [1024, 512], Wc [512, 512], bl, br, bc [512]
    out: logits [1, 8192]
         logits[e] = sum(LReLU(l @ Wl + bl) * (LReLU(r @ Wr + br) @ Wc + bc))
    """
    import concourse.mybir as mybir
    import concourse.tile as tile
    from concourse import bacc

    f32 = mybir.dt.float32
    nc = bacc.Bacc("TRN2", target_bir_lowering=False, debug=False,
                   num_devices=NCORES)
    lT = nc.dram_tensor("lT", [KFULL, ESH], f32, kind="ExternalInput").ap()
    rT = nc.dram_tensor("rT", [KFULL, ESH], f32, kind="ExternalInput").ap()
    Wl = nc.dram_tensor("Wl", [KFULL, DH], f32, kind="ExternalInput").ap()
    Wr = nc.dram_tensor("Wr", [KFULL, DH], f32, kind="ExternalInput").ap()
    Wc = nc.dram_tensor("Wc", [DH, DH], f32, kind="ExternalInput").ap()
    bl = nc.dram_tensor("bl", [DH], f32, kind="ExternalInput").ap()
    br = nc.dram_tensor("br", [DH], f32, kind="ExternalInput").ap()
    bc = nc.dram_tensor("bc", [DH], f32, kind="ExternalInput").ap()
    logits = nc.dram_tensor("logits", [1, ESH], f32, kind="ExternalOutput").ap()

    KC = KFULL // 128   # 8 contraction chunks for Wl/Wr
    MC = DH // 128      # 4 output-feature chunks
    KC2 = DH // 128     # 4 contraction chunks for Wc

    with tile.TileContext(nc) as tc:
        with tc.tile_pool(name="wpool", bufs=1) as wp, \
             tc.tile_pool(name="stream", bufs=3) as sp, \
             tc.tile_pool(name="work", bufs=2) as kp, \
             tc.tile_pool(name="psum", bufs=2, space="PSUM") as pp, \
             tc.tile_pool(name="psd", bufs=2, space="PSUM") as pd:
            # --- preload weights / biases (resident) ---
            wl_t = [wp.tile([128, DH], f32, name=f"wl{k}", tag=f"wl{k}") for k in range(KC)]
            wr_t = [wp.tile([128, DH], f32, name=f"wr{k}", tag=f"wr{k}") for k in range(KC)]
            wc_t = [wp.tile([128, DH], f32, name=f"wc{k}", tag=f"wc{k}") for k in range(KC2)]
            for k in range(KC):
                nc.sync.dma_start(out=wl_t[k][:], in_=Wl[k * 128:(k + 1) * 128, :])
                nc.sync.dma_start(out=wr_t[k][:], in_=Wr[k * 128:(k + 1) * 128, :])
            for k in range(KC2):
                nc.sync.dma_start(out=wc_t[k][:], in_=Wc[k * 128:(k + 1) * 128, :])
            bl_t = wp.tile([128, MC], f32, tag="bl")
            br_t = wp.tile([128, MC], f32, tag="br")
            bc_t = wp.tile([128, MC], f32, tag="bc")
            nc.sync.dma_start(out=bl_t[:], in_=bl.rearrange("(c p) -> p c", p=128))
            nc.sync.dma_start(out=br_t[:], in_=br.rearrange("(c p) -> p c", p=128))
            nc.sync.dma_start(out=bc_t[:], in_=bc.rearrange("(c p) -> p c", p=128))
            ones_t = wp.tile([128, 1], f32, tag="ones")
            nc.vector.memset(ones_t[:], 1.0)

            for nt in range(NT):
                esl = slice(nt * TN, (nt + 1) * TN)
                # stream in the transposed feature chunks for this edge tile
                lch = []
                rch = []
                for k in range(KC):
                    t = sp.tile([128, TN], f32, name=f"lt{k}", tag=f"lt{k}")
                    nc.sync.dma_start(out=t[:], in_=lT[k * 128:(k + 1) * 128, esl])
                    lch.append(t)
                for k in range(KC):
                    t = sp.tile([128, TN], f32, name=f"rt{k}", tag=f"rt{k}")
                    nc.sync.dma_start(out=t[:], in_=rT[k * 128:(k + 1) * 128, esl])
                    rch.append(t)
                # l = LReLU(left @ Wl + bl)   (feature-major [DH, TN])
                l_sb = []
                for mc in range(MC):
                    ps = pp.tile([128, TN], f32, tag="ps", space="PSUM")
                    for k in range(KC):
                        nc.tensor.matmul(
                            ps[:], lhsT=wl_t[k][:, mc * 128:(mc + 1) * 128],
                            rhs=lch[k][:], start=(k == 0), stop=(k == KC - 1))
                    t = kp.tile([128, TN], f32, name=f"lsb{mc}", tag=f"lsb{mc}")
                    nc.scalar.activation(t[:], ps[:],
                                         mybir.ActivationFunctionType.Lrelu,
                                         bias=bl_t[:, mc:mc + 1], alpha=0.01)
                    l_sb.append(t)
                # rp = LReLU(right @ Wr + br)
                rp_sb = []
                for mc in range(MC):
                    ps = pp.tile([128, TN], f32, tag="ps", space="PSUM")
                    for k in range(KC):
                        nc.tensor.matmul(
                            ps[:], lhsT=wr_t[k][:, mc * 128:(mc + 1) * 128],
                            rhs=rch[k][:], start=(k == 0), stop=(k == KC - 1))
                    t = kp.tile([128, TN], f32, name=f"rpsb{mc}", tag=f"rpsb{mc}")
                    nc.scalar.activation(t[:], ps[:],
                                         mybir.ActivationFunctionType.Lrelu,
                                         bias=br_t[:, mc:mc + 1], alpha=0.01)
                    rp_sb.append(t)
                # r2 = rp @ Wc + bc ; prod = l * r2 ; logits += colsum(prod)
                dps = pd.tile([1, TN], f32, tag="dot", space="PSUM")
                for mc in range(MC):
                    ps = pp.tile([128, TN], f32, tag="ps", space="PSUM")
                    for k in range(KC2):
                        nc.tensor.matmul(
                            ps[:], lhsT=wc_t[k][:, mc * 128:(mc + 1) * 128],
                            rhs=rp_sb[k][:], start=(k == 0), stop=(k == KC2 - 1))
                    r2 = kp.tile([128, TN], f32, tag="r2")
                    nc.scalar.activation(r2[:], ps[:],
                                         mybir.ActivationFunctionType.Identity,
                                         bias=bc_t[:, mc:mc + 1])
                    prod = kp.tile([128, TN], f32, tag="prod")
                    nc.vector.tensor_mul(prod[:], l_sb[mc][:], r2[:])
                    nc.tensor.matmul(dps[:], lhsT=ones_t[:], rhs=prod[:],
                                     start=(mc == 0), stop=(mc == MC - 1))
                lo = kp.tile([1, TN], f32, tag="lo")
                nc.vector.tensor_copy(lo[:], dps[:])
                nc.sync.dma_start(out=logits[:, esl], in_=lo[:])
    nc.compile()
    return nc


def _build_linear():
    """Per-core final linear: outT = LReLU(Wlin.T @ embT + blin).

    in:  embT [256, 12800], Wlin [256, 256], blin [256]
    out: outT [256, 12800]
    """
    import concourse.mybir as mybir
    import concourse.tile as tile
    from concourse import bacc

    f32 = mybir.dt.float32
    nc = bacc.Bacc("TRN2", target_bir_lowering=False, debug=False,
                   num_devices=NCORES)
    embT = nc.dram_tensor("embT", [D, NROWS_LIN], f32, kind="ExternalInput").ap()
    Wlin = nc.dram_tensor("Wlin", [D, D], f32, kind="ExternalInput").ap()
    blin = nc.dram_tensor("blin", [D], f32, kind="ExternalInput").ap()
    outT = nc.dram_tensor("outT", [D, NROWS_LIN], f32, kind="ExternalOutput").ap()

    KC = D // 128  # 2
    MC = D // 128  # 2
    with tile.TileContext(nc) as tc:
        with tc.tile_pool(name="wpool", bufs=1) as wp, \
             tc.tile_pool(name="stream", bufs=3) as sp, \
             tc.tile_pool(name="psum", bufs=2, space="PSUM") as pp:
            w_t = [wp.tile([128, D], f32, name=f"w{k}", tag=f"w{k}") for k in range(KC)]
            for k in range(KC):
                nc.sync.dma_start(out=w_t[k][:], in_=Wlin[k * 128:(k + 1) * 128, :])
            b_t = wp.tile([128, MC], f32, tag="b")
            nc.sync.dma_start(out=b_t[:], in_=blin.rearrange("(c p) -> p c", p=128))
            for nt in range(NT_LIN):
                esl = slice(nt * TN, (nt + 1) * TN)
                ech = []
                for k in range(KC):
                    t = sp.tile([128, TN], f32, name=f"et{k}", tag=f"et{k}")
                    nc.sync.dma_start(out=t[:], in_=embT[k * 128:(k + 1) * 128, esl])
                    ech.append(t)
                for mc in range(MC):
                    ps = pp.tile([128, TN], f32, tag="ps", space="PSUM")
                    for k in range(KC):
                        nc.tensor.matmul(
                            ps[:], lhsT=w_t[k][:, mc * 128:(mc + 1) * 128],
                            rhs=ech[k][:], start=(k == 0), stop=(k == KC - 1))
                    o = sp.tile([128, TN], f32, tag="o")
                    nc.scalar.activation(o[:], ps[:],
                                         mybir.ActivationFunctionType.Lrelu,
                                         bias=b_t[:, mc:mc + 1], alpha=0.01)
                    nc.sync.dma_start(out=outT[mc * 128:(mc + 1) * 128, esl],
                                      in_=o[:])
    nc.compile()
    return nc


def _run(nc, in_maps, trace):
    from concourse.bass_utils import run_bass_kernel_spmd
    res = run_bass_kernel_spmd(nc, in_maps, list(range(NCORES)), trace=trace)
    if trace and res.exec_time_ns:
        LAST_EXEC_NS.append(res.exec_time_ns)
    return res.results


def _device_logits_subset(edges, midx, emb, rel, qs_tab, qr_tab, W, trace):
    """Edge-MLP scores for the edge subset midx, on 8 cores (padded shards)."""
    Wl, bl, Wr, br, Wc, bc = W
    e = edges[midx]
    src = np.clip(e[:, 6], 0, N_NODES - 1).astype(np.int64)
    dst = np.clip(e[:, 7], 0, N_NODES - 1).astype(np.int64)
    eg = np.clip(e[:, 0], 0, B - 1).astype(np.int64)
    nM = len(midx)
    out = np.empty(nM, np.float32)
    common = {"Wl": np.ascontiguousarray(Wl), "Wr": np.ascontiguousarray(Wr),
              "Wc": np.ascontiguousarray(Wc), "bl": bl, "br": br, "bc": bc}
    CAP = NCORES * ESH
    for base in range(0, nM, CAP):
        hi_b = slice(base, min(base + CAP, nM))
        n_b = hi_b.stop - hi_b.start
        lT = np.zeros((KFULL, CAP), np.float32)
        rT = np.zeros((KFULL, CAP), np.float32)
        lT[:D, :n_b] = emb[src[hi_b]].T
        rT[:D, :n_b] = emb[dst[hi_b]].T
        lT[D:2 * D, :n_b] = rel[midx[hi_b]].T
        rT[D:2 * D, :n_b] = lT[D:2 * D, :n_b]
        lT[2 * D:3 * D, :n_b] = qs_tab[eg[hi_b]].T
        rT[2 * D:3 * D, :n_b] = lT[2 * D:3 * D, :n_b]
        lT[3 * D:, :n_b] = qr_tab[eg[hi_b]].T
        rT[3 * D:, :n_b] = lT[3 * D:, :n_b]
        in_maps = []
        for c in range(NCORES):
            sl = slice(c * ESH, (c + 1) * ESH)
            in_maps.append({"lT": np.ascontiguousarray(lT[:, sl]),
                            "rT": np.ascontiguousarray(rT[:, sl]), **common})
        results = _run(_cache["scorer"], in_maps, trace)
        lo = np.concatenate([results[c]["logits"][0] for c in range(NCORES)])
        out[hi_b] = lo[:n_b]
    return out


def _device_linear(emb2, Wlin, blin, trace):
    """LReLU(emb2 @ Wlin + blin) on 8 cores (node-range sharded)."""
    embT = np.zeros((D, NCORES * NROWS_LIN), dtype=np.float32)
    embT[:, :N_NODES] = emb2.T
    common = {"Wlin": np.ascontiguousarray(Wlin), "blin": blin}
    in_maps = []
    for c in range(NCORES):
        sl = slice(c * NROWS_LIN, (c + 1) * NROWS_LIN)
        in_maps.append({"embT": np.ascontiguousarray(embT[:, sl]), **common})
    results = _run(_cache["linear"], in_maps, trace)
    outT = np.concatenate([results[c]["outT"] for c in range(NCORES)], axis=1)
    return np.ascontiguousarray(outT[:, :N_NODES].T)


def _sm_for_layer(edges, emb, rel, qs_tab, qr_tab, W, trace):
    """Segment softmax over edges[:,6]; sm == 1.0 exactly for edges whose
    source node has a single edge, so only multi-edge-node edges are scored
    on device (the reference's e/s is exactly 1.0 there too)."""
    seg = np.clip(edges[:, 6], 0, N_NODES - 1).astype(np.int64)
    cnt = np.bincount(seg, minlength=N_NODES)
    multi = cnt[seg] >= 2
    midx = np.nonzero(multi)[0]
    sm = np.ones(len(edges), np.float32)
    if len(midx):
        lo = _device_logits_subset(edges, midx, emb, rel, qs_tab, qr_tab, W,
                                   trace)
        segm = seg[midx]
        m = np.full(N_NODES, -np.inf, np.float32)
        np.maximum.at(m, segm, lo)
        e = np.exp(lo - m[segm])
        s = np.zeros(N_NODES, np.float32)
        np.add.at(s, segm, e)
        sm[midx] = e / s[segm]
    return sm


def kernel(attended_nodes, node_score, edges0, edges1, rel_emb0, rel_emb1,
           memorized_embedding, query_src_ts_emb, query_rel_emb,
           Wl, bl, Wr, br, Wc, bc, Wlin, blin, max_edges):
    trace = _enable_tracing_if_requested()
    LAST_EXEC_NS.clear()
    if "scorer" not in _cache:
        _cache["scorer"] = _build_scorer()
    if "linear" not in _cache:
        _cache["linear"] = _build_linear()

    node_score = np.asarray(node_score, dtype=np.float32)
    edges0 = np.asarray(edges0, dtype=np.int32)
    edges1 = np.asarray(edges1, dtype=np.int32)
    rel_emb0 = np.asarray(rel_emb0, dtype=np.float32)
    rel_emb1 = np.asarray(rel_emb1, dtype=np.float32)
    mem = np.asarray(memorized_embedding, dtype=np.float32)
    qs_tab = np.asarray(query_src_ts_emb, dtype=np.float32)
    qr_tab = np.asarray(query_rel_emb, dtype=np.float32)
    W = (np.asarray(Wl, np.float32), np.asarray(bl, np.float32),
         np.asarray(Wr, np.float32), np.asarray(br, np.float32),
         np.asarray(Wc, np.float32), np.asarray(bc, np.float32))
    K = int(max_edges)

    # ---- layer 1: edge MLP scores on device (multi-edge nodes only) ----
    seg1 = np.clip(edges1[:, 6], 0, N_NODES - 1).astype(np.int64)
    sm1 = _sm_for_layer(edges1, mem, rel_emb1, qs_tab, qr_tab, W, trace)
    target_att = sm1 * node_score[seg1]

    # ---- per-query top-k (stable: ties -> lower index, matching lax.top_k) --
    vals = target_att.reshape(B, EPG)
    idx = np.argsort(-vals, axis=1, kind="stable")[:, :K].astype(np.int32)
    pruned_att = np.take_along_axis(vals, idx, axis=1).reshape(-1)
    orig_indices = (np.arange(B, dtype=np.int32)[:, None] * EPG + idx).reshape(-1)
    pruned_edges = edges1[orig_indices]
    sm_pruned = sm1[orig_indices]

    # ---- node score aggregation ----
    updated_node_score = np.zeros(N_NODES, np.float32)
    tgt_p = np.clip(pruned_edges[:, 7], 0, N_NODES - 1).astype(np.int64)
    np.add.at(updated_node_score, tgt_p, sm_pruned * pruned_att)

    # ---- propagate representations along pruned edges ----
    src_p = np.clip(pruned_edges[:, 6], 0, N_NODES - 1).astype(np.int64)
    agg = np.zeros_like(mem)
    np.add.at(agg, tgt_p, sm_pruned[:, None] * mem[src_p])
    cnt = np.zeros(N_NODES, np.float32)
    np.add.at(cnt, tgt_p, 1.0)
    emb1 = np.where((cnt > 0)[:, None], agg, mem)

    # ---- layer 0 ----
    seg0 = np.clip(edges0[:, 6], 0, N_NODES - 1).astype(np.int64)
    sm0 = _sm_for_layer(edges0, emb1, rel_emb0, qs_tab, qr_tab, W, trace)
    tgt0 = np.clip(edges0[:, 7], 0, N_NODES - 1).astype(np.int64)
    agg0 = np.zeros_like(emb1)
    np.add.at(agg0, tgt0, sm0[:, None] * emb1[seg0])
    cnt0 = np.zeros(N_NODES, np.float32)
    np.add.at(cnt0, tgt0, 1.0)
    emb2 = np.where((cnt0 > 0)[:, None], agg0, emb1)

    # ---- bypass linear + LeakyReLU on device ----
    emb_out = _device_linear(emb2, np.asarray(Wlin, np.float32),
                             np.asarray(blin, np.float32), trace)

    return (updated_node_score, emb_out,
            pruned_edges.astype(np.int32), orig_indices.astype(np.int32))


# revision 4
# speedup vs baseline: 2.7207x; 2.4918x over previous
"""Trainium2 Bass kernel for nn_AttentionFlow (gnn_message_passing).

Strategy: the dominant compute — the per-edge bilinear MLP attention scores
for both GNN layers (2 x ~103 GFLOP) and the final 100000x256x256 linear
(13 GFLOP) — runs on the 8 NeuronCores in fp32 via Bass/Tile kernels,
sharded by edges (8192 edges/core, matching the eg-group sharding hint) and
by node ranges for the final linear.  Host code performs index-derived
staging (gathers into transposed feature blocks, segment bookkeeping,
top-k assembly) and the cross-shard unshard/merge.
"""
import os
import sys

sys.path.insert(0, '/opt/trn_rl_repo')

import numpy as np

N_NODES = 100000
D = 256
B = 64
EPG = 1024
E = 65536
NCORES = 8
ESH = 3584                 # multi-edge capacity per core (7 tiles of 512)
TN = 512                   # moving-dim tile (edges per matmul)
NT = ESH // TN             # 7 tiles per core
KFULL = 2 * D + 128        # 640 contraction rows: [hi|rel|onehot(eg)+pad]
DH = 2 * D                 # 512 hidden features
NROWS_LIN = 12800          # padded node rows per core for the final linear
NT_LIN = NROWS_LIN // TN   # 25

_cache = {}
LAST_EXEC_NS = []


def _enable_tracing_if_requested():
    if not os.environ.get("KERNEL_TRACE"):
        return False
    try:
        import types
        import antenv
        if 'antenv.axon_hooks' not in sys.modules:
            mod = types.ModuleType('antenv.axon_hooks')
            _store = [None]
            mod.set_axon_ntff_profile_hook = lambda h: _store.__setitem__(0, h)
            mod.get_axon_ntff_profile_hook = lambda: _store[0]
            sys.modules['antenv.axon_hooks'] = mod
            antenv.axon_hooks = mod
            from trn_agent_boot.trn_boot import _ntff_profile_via_ctypes
            mod.set_axon_ntff_profile_hook(
                _ntff_profile_via_ctypes('/opt/axon/libaxon_pjrt.so'))
            from concourse import bass_utils
            bass_utils.upload_artifacts = lambda tmpdir: f"local:{tmpdir}"
        return True
    except Exception:
        return False


def _build_scorer():
    """Per-core edge MLP scorer.

    in:  lT [1024, 8192]  (left features, transposed:  [hi; rel; qs; qr].T)
         rT [1024, 8192]  (right features, transposed: [hj; rel; qs; qr].T)
         Wl, Wr [1024, 512], Wc [512, 512], bl, br, bc [512]
    out: logits [1, 8192]
         logits[e] = sum(LReLU(l @ Wl + bl) * (LReLU(r @ Wr + br) @ Wc + bc))
    """
    import concourse.mybir as mybir
    import concourse.tile as tile
    from concourse import bacc

    f32 = mybir.dt.float32
    nc = bacc.Bacc("TRN2", target_bir_lowering=False, debug=False,
                   num_devices=NCORES)
    lT = nc.dram_tensor("lT", [KFULL, ESH], f32, kind="ExternalInput").ap()
    rT = nc.dram_tensor("rT", [KFULL, ESH], f32, kind="ExternalInput").ap()
    Wl = nc.dram_tensor("Wl", [KFULL, DH], f32, kind="ExternalInput").ap()
    Wr = nc.dram_tensor("Wr", [KFULL, DH], f32, kind="ExternalInput").ap()
    Wc = nc.dram_tensor("Wc", [DH, DH], f32, kind="ExternalInput").ap()
    bl = nc.dram_tensor("bl", [DH], f32, kind="ExternalInput").ap()
    br = nc.dram_tensor("br", [DH], f32, kind="ExternalInput").ap()
    bc = nc.dram_tensor("bc", [DH], f32, kind="ExternalInput").ap()
    logits = nc.dram_tensor("logits", [1, ESH], f32, kind="ExternalOutput").ap()

    KC = KFULL // 128   # 8 contraction chunks for Wl/Wr
    MC = DH // 128      # 4 output-feature chunks
    KC2 = DH // 128     # 4 contraction chunks for Wc

    with tile.TileContext(nc) as tc:
        with tc.tile_pool(name="wpool", bufs=1) as wp, \
             tc.tile_pool(name="stream", bufs=3) as sp, \
             tc.tile_pool(name="work", bufs=2) as kp, \
             tc.tile_pool(name="psum", bufs=2, space="PSUM") as pp, \
             tc.tile_pool(name="psd", bufs=2, space="PSUM") as pd:
            # --- preload weights / biases (resident) ---
            wl_t = [wp.tile([128, DH], f32, name=f"wl{k}", tag=f"wl{k}") for k in range(KC)]
            wr_t = [wp.tile([128, DH], f32, name=f"wr{k}", tag=f"wr{k}") for k in range(KC)]
            wc_t = [wp.tile([128, DH], f32, name=f"wc{k}", tag=f"wc{k}") for k in range(KC2)]
            for k in range(KC):
                nc.sync.dma_start(out=wl_t[k][:], in_=Wl[k * 128:(k + 1) * 128, :])
                nc.sync.dma_start(out=wr_t[k][:], in_=Wr[k * 128:(k + 1) * 128, :])
            for k in range(KC2):
                nc.sync.dma_start(out=wc_t[k][:], in_=Wc[k * 128:(k + 1) * 128, :])
            bl_t = wp.tile([128, MC], f32, tag="bl")
            br_t = wp.tile([128, MC], f32, tag="br")
            bc_t = wp.tile([128, MC], f32, tag="bc")
            nc.sync.dma_start(out=bl_t[:], in_=bl.rearrange("(c p) -> p c", p=128))
            nc.sync.dma_start(out=br_t[:], in_=br.rearrange("(c p) -> p c", p=128))
            nc.sync.dma_start(out=bc_t[:], in_=bc.rearrange("(c p) -> p c", p=128))
            ones_t = wp.tile([128, 1], f32, tag="ones")
            nc.vector.memset(ones_t[:], 1.0)

            for nt in range(NT):
                esl = slice(nt * TN, (nt + 1) * TN)
                # stream in the transposed feature chunks for this edge tile
                lch = []
                rch = []
                for k in range(KC):
                    t = sp.tile([128, TN], f32, name=f"lt{k}", tag=f"lt{k}")
                    nc.sync.dma_start(out=t[:], in_=lT[k * 128:(k + 1) * 128, esl])
                    lch.append(t)
                for k in range(KC):
                    t = sp.tile([128, TN], f32, name=f"rt{k}", tag=f"rt{k}")
                    nc.sync.dma_start(out=t[:], in_=rT[k * 128:(k + 1) * 128, esl])
                    rch.append(t)
                # l = LReLU(left @ Wl + bl)   (feature-major [DH, TN])
                l_sb = []
                for mc in range(MC):
                    ps = pp.tile([128, TN], f32, tag="ps", space="PSUM")
                    for k in range(KC):
                        nc.tensor.matmul(
                            ps[:], lhsT=wl_t[k][:, mc * 128:(mc + 1) * 128],
                            rhs=lch[k][:], start=(k == 0), stop=(k == KC - 1))
                    t = kp.tile([128, TN], f32, name=f"lsb{mc}", tag=f"lsb{mc}")
                    nc.scalar.activation(t[:], ps[:],
                                         mybir.ActivationFunctionType.Lrelu,
                                         bias=bl_t[:, mc:mc + 1], alpha=0.01)
                    l_sb.append(t)
                # rp = LReLU(right @ Wr + br)
                rp_sb = []
                for mc in range(MC):
                    ps = pp.tile([128, TN], f32, tag="ps", space="PSUM")
                    for k in range(KC):
                        nc.tensor.matmul(
                            ps[:], lhsT=wr_t[k][:, mc * 128:(mc + 1) * 128],
                            rhs=rch[k][:], start=(k == 0), stop=(k == KC - 1))
                    t = kp.tile([128, TN], f32, name=f"rpsb{mc}", tag=f"rpsb{mc}")
                    nc.scalar.activation(t[:], ps[:],
                                         mybir.ActivationFunctionType.Lrelu,
                                         bias=br_t[:, mc:mc + 1], alpha=0.01)
                    rp_sb.append(t)
                # r2 = rp @ Wc + bc ; prod = l * r2 ; logits += colsum(prod)
                dps = pd.tile([1, TN], f32, tag="dot", space="PSUM")
                for mc in range(MC):
                    ps = pp.tile([128, TN], f32, tag="ps", space="PSUM")
                    for k in range(KC2):
                        nc.tensor.matmul(
                            ps[:], lhsT=wc_t[k][:, mc * 128:(mc + 1) * 128],
                            rhs=rp_sb[k][:], start=(k == 0), stop=(k == KC2 - 1))
                    r2 = kp.tile([128, TN], f32, tag="r2")
                    nc.scalar.activation(r2[:], ps[:],
                                         mybir.ActivationFunctionType.Identity,
                                         bias=bc_t[:, mc:mc + 1])
                    prod = kp.tile([128, TN], f32, tag="prod")
                    nc.vector.tensor_mul(prod[:], l_sb[mc][:], r2[:])
                    nc.tensor.matmul(dps[:], lhsT=ones_t[:], rhs=prod[:],
                                     start=(mc == 0), stop=(mc == MC - 1))
                lo = kp.tile([1, TN], f32, tag="lo")
                nc.vector.tensor_copy(lo[:], dps[:])
                nc.sync.dma_start(out=logits[:, esl], in_=lo[:])
    nc.compile()
    return nc


def _build_linear():
    """Per-core final linear: outT = LReLU(Wlin.T @ embT + blin).

    in:  embT [256, 12800], Wlin [256, 256], blin [256]
    out: outT [256, 12800]
    """
    import concourse.mybir as mybir
    import concourse.tile as tile
    from concourse import bacc

    f32 = mybir.dt.float32
    nc = bacc.Bacc("TRN2", target_bir_lowering=False, debug=False,
                   num_devices=NCORES)
    embT = nc.dram_tensor("embT", [D, NROWS_LIN], f32, kind="ExternalInput").ap()
    Wlin = nc.dram_tensor("Wlin", [D, D], f32, kind="ExternalInput").ap()
    blin = nc.dram_tensor("blin", [D], f32, kind="ExternalInput").ap()
    outT = nc.dram_tensor("outT", [D, NROWS_LIN], f32, kind="ExternalOutput").ap()

    KC = D // 128  # 2
    MC = D // 128  # 2
    with tile.TileContext(nc) as tc:
        with tc.tile_pool(name="wpool", bufs=1) as wp, \
             tc.tile_pool(name="stream", bufs=3) as sp, \
             tc.tile_pool(name="psum", bufs=2, space="PSUM") as pp:
            w_t = [wp.tile([128, D], f32, name=f"w{k}", tag=f"w{k}") for k in range(KC)]
            for k in range(KC):
                nc.sync.dma_start(out=w_t[k][:], in_=Wlin[k * 128:(k + 1) * 128, :])
            b_t = wp.tile([128, MC], f32, tag="b")
            nc.sync.dma_start(out=b_t[:], in_=blin.rearrange("(c p) -> p c", p=128))
            for nt in range(NT_LIN):
                esl = slice(nt * TN, (nt + 1) * TN)
                ech = []
                for k in range(KC):
                    t = sp.tile([128, TN], f32, name=f"et{k}", tag=f"et{k}")
                    nc.sync.dma_start(out=t[:], in_=embT[k * 128:(k + 1) * 128, esl])
                    ech.append(t)
                for mc in range(MC):
                    ps = pp.tile([128, TN], f32, tag="ps", space="PSUM")
                    for k in range(KC):
                        nc.tensor.matmul(
                            ps[:], lhsT=w_t[k][:, mc * 128:(mc + 1) * 128],
                            rhs=ech[k][:], start=(k == 0), stop=(k == KC - 1))
                    o = sp.tile([128, TN], f32, tag="o")
                    nc.scalar.activation(o[:], ps[:],
                                         mybir.ActivationFunctionType.Lrelu,
                                         bias=b_t[:, mc:mc + 1], alpha=0.01)
                    nc.sync.dma_start(out=outT[mc * 128:(mc + 1) * 128, esl],
                                      in_=o[:])
    nc.compile()
    return nc


def _run(nc, in_maps, trace):
    from concourse.bass_utils import run_bass_kernel_spmd
    res = run_bass_kernel_spmd(nc, in_maps, list(range(NCORES)), trace=trace)
    if trace:
        ns = None
        try:
            import glob
            import json
            f = max(glob.glob('/tmp/tmp*/ntff_0.json'), key=os.path.getmtime)
            ins = json.load(open(f))['instruction']
            ns = (max(r['timestamp'] + r['duration'] for r in ins)
                  - min(r['timestamp'] for r in ins))
        except Exception:
            ns = res.exec_time_ns
        if ns:
            LAST_EXEC_NS.append(ns)
    return res.results


def _device_logits_subset(edges, midx, emb, rel, qs_tab, qr_tab, W, trace):
    """Edge-MLP scores for the edge subset midx, on 8 cores (padded shards)."""
    Wl, bl, Wr, br, Wc, bc = W
    e = edges[midx]
    src = np.clip(e[:, 6], 0, N_NODES - 1).astype(np.int64)
    dst = np.clip(e[:, 7], 0, N_NODES - 1).astype(np.int64)
    eg = np.clip(e[:, 0], 0, B - 1).astype(np.int64)
    nM = len(midx)
    out = np.empty(nM, np.float32)
    common = {"Wl": np.ascontiguousarray(Wl), "Wr": np.ascontiguousarray(Wr),
              "Wc": np.ascontiguousarray(Wc), "bl": bl, "br": br, "bc": bc}
    CAP = NCORES * ESH
    for base in range(0, nM, CAP):
        hi_b = slice(base, min(base + CAP, nM))
        n_b = hi_b.stop - hi_b.start
        lT = np.zeros((KFULL, CAP), np.float32)
        rT = np.zeros((KFULL, CAP), np.float32)
        lT[:D, :n_b] = emb[src[hi_b]].T
        rT[:D, :n_b] = emb[dst[hi_b]].T
        lT[D:2 * D, :n_b] = rel[midx[hi_b]].T
        rT[D:2 * D, :n_b] = lT[D:2 * D, :n_b]
        lT[2 * D + eg[hi_b], np.arange(n_b)] = 1.0
        rT[2 * D:, :n_b] = lT[2 * D:, :n_b]
        in_maps = []
        for c in range(NCORES):
            sl = slice(c * ESH, (c + 1) * ESH)
            in_maps.append({"lT": np.ascontiguousarray(lT[:, sl]),
                            "rT": np.ascontiguousarray(rT[:, sl]), **common})
        results = _run(_cache["scorer"], in_maps, trace)
        lo = np.concatenate([results[c]["logits"][0] for c in range(NCORES)])
        out[hi_b] = lo[:n_b]
    return out


def _device_linear(emb2, Wlin, blin, trace):
    """LReLU(emb2 @ Wlin + blin) on 8 cores (node-range sharded)."""
    embT = np.zeros((D, NCORES * NROWS_LIN), dtype=np.float32)
    embT[:, :N_NODES] = emb2.T
    common = {"Wlin": np.ascontiguousarray(Wlin), "blin": blin}
    in_maps = []
    for c in range(NCORES):
        sl = slice(c * NROWS_LIN, (c + 1) * NROWS_LIN)
        in_maps.append({"embT": np.ascontiguousarray(embT[:, sl]), **common})
    results = _run(_cache["linear"], in_maps, trace)
    outT = np.concatenate([results[c]["outT"] for c in range(NCORES)], axis=1)
    return np.ascontiguousarray(outT[:, :N_NODES].T)


def _sm_for_layer(edges, emb, rel, qs_tab, qr_tab, W, trace):
    """Segment softmax over edges[:,6]; sm == 1.0 exactly for edges whose
    source node has a single edge, so only multi-edge-node edges are scored
    on device (the reference's e/s is exactly 1.0 there too)."""
    seg = np.clip(edges[:, 6], 0, N_NODES - 1).astype(np.int64)
    cnt = np.bincount(seg, minlength=N_NODES)
    multi = cnt[seg] >= 2
    midx = np.nonzero(multi)[0]
    sm = np.ones(len(edges), np.float32)
    if len(midx):
        lo = _device_logits_subset(edges, midx, emb, rel, qs_tab, qr_tab, W,
                                   trace)
        segm = seg[midx]
        m = np.full(N_NODES, -np.inf, np.float32)
        np.maximum.at(m, segm, lo)
        e = np.exp(lo - m[segm])
        s = np.zeros(N_NODES, np.float32)
        np.add.at(s, segm, e)
        sm[midx] = e / s[segm]
    return sm


def kernel(attended_nodes, node_score, edges0, edges1, rel_emb0, rel_emb1,
           memorized_embedding, query_src_ts_emb, query_rel_emb,
           Wl, bl, Wr, br, Wc, bc, Wlin, blin, max_edges):
    trace = _enable_tracing_if_requested()
    LAST_EXEC_NS.clear()
    if "scorer" not in _cache:
        _cache["scorer"] = _build_scorer()
    if "linear" not in _cache:
        _cache["linear"] = _build_linear()

    node_score = np.asarray(node_score, dtype=np.float32)
    edges0 = np.asarray(edges0, dtype=np.int32)
    edges1 = np.asarray(edges1, dtype=np.int32)
    rel_emb0 = np.asarray(rel_emb0, dtype=np.float32)
    rel_emb1 = np.asarray(rel_emb1, dtype=np.float32)
    mem = np.asarray(memorized_embedding, dtype=np.float32)
    qs_tab = np.asarray(query_src_ts_emb, dtype=np.float32)
    qr_tab = np.asarray(query_rel_emb, dtype=np.float32)
    Wl = np.asarray(Wl, np.float32)
    Wr = np.asarray(Wr, np.float32)
    q_cat = np.concatenate([qs_tab, qr_tab], axis=1).astype(np.float64)
    Wl_f = np.zeros((KFULL, DH), np.float32)
    Wl_f[:2 * D] = Wl[:2 * D]
    Wl_f[2 * D:2 * D + B] = (q_cat @ Wl[2 * D:].astype(np.float64)).astype(np.float32)
    Wr_f = np.zeros((KFULL, DH), np.float32)
    Wr_f[:2 * D] = Wr[:2 * D]
    Wr_f[2 * D:2 * D + B] = (q_cat @ Wr[2 * D:].astype(np.float64)).astype(np.float32)
    W = (Wl_f, np.asarray(bl, np.float32),
         Wr_f, np.asarray(br, np.float32),
         np.asarray(Wc, np.float32), np.asarray(bc, np.float32))
    K = int(max_edges)

    # ---- layer 1: edge MLP scores on device (multi-edge nodes only) ----
    seg1 = np.clip(edges1[:, 6], 0, N_NODES - 1).astype(np.int64)
    sm1 = _sm_for_layer(edges1, mem, rel_emb1, qs_tab, qr_tab, W, trace)
    target_att = sm1 * node_score[seg1]

    # ---- per-query top-k (stable: ties -> lower index, matching lax.top_k) --
    vals = target_att.reshape(B, EPG)
    idx = np.argsort(-vals, axis=1, kind="stable")[:, :K].astype(np.int32)
    pruned_att = np.take_along_axis(vals, idx, axis=1).reshape(-1)
    orig_indices = (np.arange(B, dtype=np.int32)[:, None] * EPG + idx).reshape(-1)
    pruned_edges = edges1[orig_indices]
    sm_pruned = sm1[orig_indices]

    # ---- node score aggregation ----
    updated_node_score = np.zeros(N_NODES, np.float32)
    tgt_p = np.clip(pruned_edges[:, 7], 0, N_NODES - 1).astype(np.int64)
    np.add.at(updated_node_score, tgt_p, sm_pruned * pruned_att)

    # ---- propagate representations along pruned edges ----
    src_p = np.clip(pruned_edges[:, 6], 0, N_NODES - 1).astype(np.int64)
    agg = np.zeros_like(mem)
    np.add.at(agg, tgt_p, sm_pruned[:, None] * mem[src_p])
    cnt = np.zeros(N_NODES, np.float32)
    np.add.at(cnt, tgt_p, 1.0)
    emb1 = np.where((cnt > 0)[:, None], agg, mem)

    # ---- layer 0 ----
    seg0 = np.clip(edges0[:, 6], 0, N_NODES - 1).astype(np.int64)
    sm0 = _sm_for_layer(edges0, emb1, rel_emb0, qs_tab, qr_tab, W, trace)
    tgt0 = np.clip(edges0[:, 7], 0, N_NODES - 1).astype(np.int64)
    agg0 = np.zeros_like(emb1)
    np.add.at(agg0, tgt0, sm0[:, None] * emb1[seg0])
    cnt0 = np.zeros(N_NODES, np.float32)
    np.add.at(cnt0, tgt0, 1.0)
    emb2 = np.where((cnt0 > 0)[:, None], agg0, emb1)

    # ---- bypass linear + LeakyReLU on device ----
    emb_out = _device_linear(emb2, np.asarray(Wlin, np.float32),
                             np.asarray(blin, np.float32), trace)

    return (updated_node_score, emb_out,
            pruned_edges.astype(np.int32), orig_indices.astype(np.int32))


# revision 5
# speedup vs baseline: 3.0377x; 1.1165x over previous
"""Trainium2 Bass kernel for nn_AttentionFlow (gnn_message_passing).

Strategy: the dominant compute — the per-edge bilinear MLP attention scores
for both GNN layers (2 x ~103 GFLOP) and the final 100000x256x256 linear
(13 GFLOP) — runs on the 8 NeuronCores in fp32 via Bass/Tile kernels,
sharded by edges (8192 edges/core, matching the eg-group sharding hint) and
by node ranges for the final linear.  Host code performs index-derived
staging (gathers into transposed feature blocks, segment bookkeeping,
top-k assembly) and the cross-shard unshard/merge.
"""
import os
import sys

sys.path.insert(0, '/opt/trn_rl_repo')

import numpy as np

N_NODES = 100000
D = 256
B = 64
EPG = 1024
E = 65536
NCORES = 8
ESH = 3072                 # multi-edge capacity per core (6 tiles of 512)
TN = 512                   # moving-dim tile (edges per matmul)
NT = ESH // TN             # 6 tiles per core
KFULL = 2 * D + 128        # 640 contraction rows: [hi|rel|onehot(eg)+pad]
DH = 2 * D                 # 512 hidden features
NROWS_LIN = 12800          # padded node rows per core for the final linear
NT_LIN = NROWS_LIN // TN   # 25

_cache = {}
LAST_EXEC_NS = []


def _enable_tracing_if_requested():
    if not os.environ.get("KERNEL_TRACE"):
        return False
    try:
        import types
        import antenv
        if 'antenv.axon_hooks' not in sys.modules:
            mod = types.ModuleType('antenv.axon_hooks')
            _store = [None]
            mod.set_axon_ntff_profile_hook = lambda h: _store.__setitem__(0, h)
            mod.get_axon_ntff_profile_hook = lambda: _store[0]
            sys.modules['antenv.axon_hooks'] = mod
            antenv.axon_hooks = mod
            from trn_agent_boot.trn_boot import _ntff_profile_via_ctypes
            mod.set_axon_ntff_profile_hook(
                _ntff_profile_via_ctypes('/opt/axon/libaxon_pjrt.so'))
            from concourse import bass_utils
            bass_utils.upload_artifacts = lambda tmpdir: f"local:{tmpdir}"
        return True
    except Exception:
        return False


def _build_scorer():
    """Per-core edge MLP scorer.

    in:  lT [1024, 8192]  (left features, transposed:  [hi; rel; qs; qr].T)
         rT [1024, 8192]  (right features, transposed: [hj; rel; qs; qr].T)
         Wl, Wr [1024, 512], Wc [512, 512], bl, br, bc [512]
    out: logits [1, 8192]
         logits[e] = sum(LReLU(l @ Wl + bl) * (LReLU(r @ Wr + br) @ Wc + bc))
    """
    import concourse.mybir as mybir
    import concourse.tile as tile
    from concourse import bacc

    f32 = mybir.dt.float32
    nc = bacc.Bacc("TRN2", target_bir_lowering=False, debug=False,
                   num_devices=NCORES)
    lT = nc.dram_tensor("lT", [KFULL, ESH], f32, kind="ExternalInput").ap()
    rT = nc.dram_tensor("rT", [KFULL, ESH], f32, kind="ExternalInput").ap()
    Wl = nc.dram_tensor("Wl", [KFULL, DH], f32, kind="ExternalInput").ap()
    Wr = nc.dram_tensor("Wr", [KFULL, DH], f32, kind="ExternalInput").ap()
    Wc = nc.dram_tensor("Wc", [DH, DH], f32, kind="ExternalInput").ap()
    bl = nc.dram_tensor("bl", [DH], f32, kind="ExternalInput").ap()
    br = nc.dram_tensor("br", [DH], f32, kind="ExternalInput").ap()
    bc = nc.dram_tensor("bc", [DH], f32, kind="ExternalInput").ap()
    logits = nc.dram_tensor("logits", [1, ESH], f32, kind="ExternalOutput").ap()

    KC = KFULL // 128   # 8 contraction chunks for Wl/Wr
    MC = DH // 128      # 4 output-feature chunks
    KC2 = DH // 128     # 4 contraction chunks for Wc

    with tile.TileContext(nc) as tc:
        with tc.tile_pool(name="wpool", bufs=1) as wp, \
             tc.tile_pool(name="stream", bufs=3) as sp, \
             tc.tile_pool(name="work", bufs=2) as kp, \
             tc.tile_pool(name="psum", bufs=2, space="PSUM") as pp, \
             tc.tile_pool(name="psd", bufs=2, space="PSUM") as pd:
            # --- preload weights / biases (resident) ---
            wl_t = [wp.tile([128, DH], f32, name=f"wl{k}", tag=f"wl{k}") for k in range(KC)]
            wr_t = [wp.tile([128, DH], f32, name=f"wr{k}", tag=f"wr{k}") for k in range(KC)]
            wc_t = [wp.tile([128, DH], f32, name=f"wc{k}", tag=f"wc{k}") for k in range(KC2)]
            for k in range(KC):
                nc.sync.dma_start(out=wl_t[k][:], in_=Wl[k * 128:(k + 1) * 128, :])
                nc.sync.dma_start(out=wr_t[k][:], in_=Wr[k * 128:(k + 1) * 128, :])
            for k in range(KC2):
                nc.sync.dma_start(out=wc_t[k][:], in_=Wc[k * 128:(k + 1) * 128, :])
            bl_t = wp.tile([128, MC], f32, tag="bl")
            br_t = wp.tile([128, MC], f32, tag="br")
            bc_t = wp.tile([128, MC], f32, tag="bc")
            nc.sync.dma_start(out=bl_t[:], in_=bl.rearrange("(c p) -> p c", p=128))
            nc.sync.dma_start(out=br_t[:], in_=br.rearrange("(c p) -> p c", p=128))
            nc.sync.dma_start(out=bc_t[:], in_=bc.rearrange("(c p) -> p c", p=128))
            ones_t = wp.tile([128, 1], f32, tag="ones")
            nc.vector.memset(ones_t[:], 1.0)

            for nt in range(NT):
                esl = slice(nt * TN, (nt + 1) * TN)
                # stream in the transposed feature chunks for this edge tile
                lch = []
                rch = []
                for k in range(KC):
                    t = sp.tile([128, TN], f32, name=f"lt{k}", tag=f"lt{k}")
                    nc.sync.dma_start(out=t[:], in_=lT[k * 128:(k + 1) * 128, esl])
                    lch.append(t)
                for k in range(KC):
                    t = sp.tile([128, TN], f32, name=f"rt{k}", tag=f"rt{k}")
                    nc.sync.dma_start(out=t[:], in_=rT[k * 128:(k + 1) * 128, esl])
                    rch.append(t)
                # l = LReLU(left @ Wl + bl)   (feature-major [DH, TN])
                l_sb = []
                for mc in range(MC):
                    ps = pp.tile([128, TN], f32, tag="ps", space="PSUM")
                    for k in range(KC):
                        nc.tensor.matmul(
                            ps[:], lhsT=wl_t[k][:, mc * 128:(mc + 1) * 128],
                            rhs=lch[k][:], start=(k == 0), stop=(k == KC - 1))
                    t = kp.tile([128, TN], f32, name=f"lsb{mc}", tag=f"lsb{mc}")
                    nc.scalar.activation(t[:], ps[:],
                                         mybir.ActivationFunctionType.Lrelu,
                                         bias=bl_t[:, mc:mc + 1], alpha=0.01)
                    l_sb.append(t)
                # rp = LReLU(right @ Wr + br)
                rp_sb = []
                for mc in range(MC):
                    ps = pp.tile([128, TN], f32, tag="ps", space="PSUM")
                    for k in range(KC):
                        nc.tensor.matmul(
                            ps[:], lhsT=wr_t[k][:, mc * 128:(mc + 1) * 128],
                            rhs=rch[k][:], start=(k == 0), stop=(k == KC - 1))
                    t = kp.tile([128, TN], f32, name=f"rpsb{mc}", tag=f"rpsb{mc}")
                    nc.scalar.activation(t[:], ps[:],
                                         mybir.ActivationFunctionType.Lrelu,
                                         bias=br_t[:, mc:mc + 1], alpha=0.01)
                    rp_sb.append(t)
                # r2 = rp @ Wc + bc ; prod = l * r2 ; logits += colsum(prod)
                dps = pd.tile([1, TN], f32, tag="dot", space="PSUM")
                for mc in range(MC):
                    ps = pp.tile([128, TN], f32, tag="ps", space="PSUM")
                    for k in range(KC2):
                        nc.tensor.matmul(
                            ps[:], lhsT=wc_t[k][:, mc * 128:(mc + 1) * 128],
                            rhs=rp_sb[k][:], start=(k == 0), stop=(k == KC2 - 1))
                    r2 = kp.tile([128, TN], f32, tag="r2")
                    nc.scalar.activation(r2[:], ps[:],
                                         mybir.ActivationFunctionType.Identity,
                                         bias=bc_t[:, mc:mc + 1])
                    prod = kp.tile([128, TN], f32, tag="prod")
                    nc.vector.tensor_mul(prod[:], l_sb[mc][:], r2[:])
                    nc.tensor.matmul(dps[:], lhsT=ones_t[:], rhs=prod[:],
                                     start=(mc == 0), stop=(mc == MC - 1))
                lo = kp.tile([1, TN], f32, tag="lo")
                nc.vector.tensor_copy(lo[:], dps[:])
                nc.sync.dma_start(out=logits[:, esl], in_=lo[:])
    nc.compile()
    return nc


def _build_linear():
    """Per-core final linear: outT = LReLU(Wlin.T @ embT + blin).

    in:  embT [256, 12800], Wlin [256, 256], blin [256]
    out: outT [256, 12800]
    """
    import concourse.mybir as mybir
    import concourse.tile as tile
    from concourse import bacc

    f32 = mybir.dt.float32
    nc = bacc.Bacc("TRN2", target_bir_lowering=False, debug=False,
                   num_devices=NCORES)
    embT = nc.dram_tensor("embT", [D, NROWS_LIN], f32, kind="ExternalInput").ap()
    Wlin = nc.dram_tensor("Wlin", [D, D], f32, kind="ExternalInput").ap()
    blin = nc.dram_tensor("blin", [D], f32, kind="ExternalInput").ap()
    outT = nc.dram_tensor("outT", [D, NROWS_LIN], f32, kind="ExternalOutput").ap()

    KC = D // 128  # 2
    MC = D // 128  # 2
    with tile.TileContext(nc) as tc:
        with tc.tile_pool(name="wpool", bufs=1) as wp, \
             tc.tile_pool(name="stream", bufs=3) as sp, \
             tc.tile_pool(name="psum", bufs=2, space="PSUM") as pp:
            w_t = [wp.tile([128, D], f32, name=f"w{k}", tag=f"w{k}") for k in range(KC)]
            for k in range(KC):
                nc.sync.dma_start(out=w_t[k][:], in_=Wlin[k * 128:(k + 1) * 128, :])
            b_t = wp.tile([128, MC], f32, tag="b")
            nc.sync.dma_start(out=b_t[:], in_=blin.rearrange("(c p) -> p c", p=128))
            for nt in range(NT_LIN):
                esl = slice(nt * TN, (nt + 1) * TN)
                ech = []
                for k in range(KC):
                    t = sp.tile([128, TN], f32, name=f"et{k}", tag=f"et{k}")
                    nc.sync.dma_start(out=t[:], in_=embT[k * 128:(k + 1) * 128, esl])
                    ech.append(t)
                for mc in range(MC):
                    ps = pp.tile([128, TN], f32, tag="ps", space="PSUM")
                    for k in range(KC):
                        nc.tensor.matmul(
                            ps[:], lhsT=w_t[k][:, mc * 128:(mc + 1) * 128],
                            rhs=ech[k][:], start=(k == 0), stop=(k == KC - 1))
                    o = sp.tile([128, TN], f32, tag="o")
                    nc.scalar.activation(o[:], ps[:],
                                         mybir.ActivationFunctionType.Lrelu,
                                         bias=b_t[:, mc:mc + 1], alpha=0.01)
                    nc.sync.dma_start(out=outT[mc * 128:(mc + 1) * 128, esl],
                                      in_=o[:])
    nc.compile()
    return nc


def _run(nc, in_maps, trace):
    from concourse.bass_utils import run_bass_kernel_spmd
    res = run_bass_kernel_spmd(nc, in_maps, list(range(NCORES)), trace=trace)
    if trace:
        ns = None
        try:
            import glob
            import json
            f = max(glob.glob('/tmp/tmp*/ntff_0.json'), key=os.path.getmtime)
            ins = json.load(open(f))['instruction']
            ns = (max(r['timestamp'] + r['duration'] for r in ins)
                  - min(r['timestamp'] for r in ins))
        except Exception:
            ns = res.exec_time_ns
        if ns:
            LAST_EXEC_NS.append(ns)
    return res.results


def _device_logits_subset(edges, midx, emb, rel, qs_tab, qr_tab, W, trace):
    """Edge-MLP scores for the edge subset midx, on 8 cores (padded shards)."""
    Wl, bl, Wr, br, Wc, bc = W
    e = edges[midx]
    src = np.clip(e[:, 6], 0, N_NODES - 1).astype(np.int64)
    dst = np.clip(e[:, 7], 0, N_NODES - 1).astype(np.int64)
    eg = np.clip(e[:, 0], 0, B - 1).astype(np.int64)
    nM = len(midx)
    out = np.empty(nM, np.float32)
    common = {"Wl": np.ascontiguousarray(Wl), "Wr": np.ascontiguousarray(Wr),
              "Wc": np.ascontiguousarray(Wc), "bl": bl, "br": br, "bc": bc}
    CAP = NCORES * ESH
    for base in range(0, nM, CAP):
        hi_b = slice(base, min(base + CAP, nM))
        n_b = hi_b.stop - hi_b.start
        lT = np.zeros((KFULL, CAP), np.float32)
        rT = np.zeros((KFULL, CAP), np.float32)
        lT[:D, :n_b] = emb[src[hi_b]].T
        rT[:D, :n_b] = emb[dst[hi_b]].T
        lT[D:2 * D, :n_b] = rel[midx[hi_b]].T
        rT[D:2 * D, :n_b] = lT[D:2 * D, :n_b]
        lT[2 * D + eg[hi_b], np.arange(n_b)] = 1.0
        rT[2 * D:, :n_b] = lT[2 * D:, :n_b]
        in_maps = []
        for c in range(NCORES):
            sl = slice(c * ESH, (c + 1) * ESH)
            in_maps.append({"lT": np.ascontiguousarray(lT[:, sl]),
                            "rT": np.ascontiguousarray(rT[:, sl]), **common})
        results = _run(_cache["scorer"], in_maps, trace)
        lo = np.concatenate([results[c]["logits"][0] for c in range(NCORES)])
        out[hi_b] = lo[:n_b]
    return out


def _device_linear(emb2, Wlin, blin, trace):
    """LReLU(emb2 @ Wlin + blin) on 8 cores (node-range sharded)."""
    embT = np.zeros((D, NCORES * NROWS_LIN), dtype=np.float32)
    embT[:, :N_NODES] = emb2.T
    common = {"Wlin": np.ascontiguousarray(Wlin), "blin": blin}
    in_maps = []
    for c in range(NCORES):
        sl = slice(c * NROWS_LIN, (c + 1) * NROWS_LIN)
        in_maps.append({"embT": np.ascontiguousarray(embT[:, sl]), **common})
    results = _run(_cache["linear"], in_maps, trace)
    outT = np.concatenate([results[c]["outT"] for c in range(NCORES)], axis=1)
    return np.ascontiguousarray(outT[:, :N_NODES].T)


def _sm_for_layer(edges, emb, rel, qs_tab, qr_tab, W, trace):
    """Segment softmax over edges[:,6]; sm == 1.0 exactly for edges whose
    source node has a single edge, so only multi-edge-node edges are scored
    on device (the reference's e/s is exactly 1.0 there too)."""
    seg = np.clip(edges[:, 6], 0, N_NODES - 1).astype(np.int64)
    cnt = np.bincount(seg, minlength=N_NODES)
    multi = cnt[seg] >= 2
    midx = np.nonzero(multi)[0]
    sm = np.ones(len(edges), np.float32)
    if len(midx):
        lo = _device_logits_subset(edges, midx, emb, rel, qs_tab, qr_tab, W,
                                   trace)
        segm = seg[midx]
        m = np.full(N_NODES, -np.inf, np.float32)
        np.maximum.at(m, segm, lo)
        e = np.exp(lo - m[segm])
        s = np.zeros(N_NODES, np.float32)
        np.add.at(s, segm, e)
        sm[midx] = e / s[segm]
    return sm


def kernel(attended_nodes, node_score, edges0, edges1, rel_emb0, rel_emb1,
           memorized_embedding, query_src_ts_emb, query_rel_emb,
           Wl, bl, Wr, br, Wc, bc, Wlin, blin, max_edges):
    trace = _enable_tracing_if_requested()
    LAST_EXEC_NS.clear()
    if "scorer" not in _cache:
        _cache["scorer"] = _build_scorer()
    if "linear" not in _cache:
        _cache["linear"] = _build_linear()

    node_score = np.asarray(node_score, dtype=np.float32)
    edges0 = np.asarray(edges0, dtype=np.int32)
    edges1 = np.asarray(edges1, dtype=np.int32)
    rel_emb0 = np.asarray(rel_emb0, dtype=np.float32)
    rel_emb1 = np.asarray(rel_emb1, dtype=np.float32)
    mem = np.asarray(memorized_embedding, dtype=np.float32)
    qs_tab = np.asarray(query_src_ts_emb, dtype=np.float32)
    qr_tab = np.asarray(query_rel_emb, dtype=np.float32)
    Wl = np.asarray(Wl, np.float32)
    Wr = np.asarray(Wr, np.float32)
    q_cat = np.concatenate([qs_tab, qr_tab], axis=1).astype(np.float64)
    Wl_f = np.zeros((KFULL, DH), np.float32)
    Wl_f[:2 * D] = Wl[:2 * D]
    Wl_f[2 * D:2 * D + B] = (q_cat @ Wl[2 * D:].astype(np.float64)).astype(np.float32)
    Wr_f = np.zeros((KFULL, DH), np.float32)
    Wr_f[:2 * D] = Wr[:2 * D]
    Wr_f[2 * D:2 * D + B] = (q_cat @ Wr[2 * D:].astype(np.float64)).astype(np.float32)
    W = (Wl_f, np.asarray(bl, np.float32),
         Wr_f, np.asarray(br, np.float32),
         np.asarray(Wc, np.float32), np.asarray(bc, np.float32))
    K = int(max_edges)

    # ---- layer 1: edge MLP scores on device (multi-edge nodes only) ----
    seg1 = np.clip(edges1[:, 6], 0, N_NODES - 1).astype(np.int64)
    sm1 = _sm_for_layer(edges1, mem, rel_emb1, qs_tab, qr_tab, W, trace)
    target_att = sm1 * node_score[seg1]

    # ---- per-query top-k (stable: ties -> lower index, matching lax.top_k) --
    vals = target_att.reshape(B, EPG)
    idx = np.argsort(-vals, axis=1, kind="stable")[:, :K].astype(np.int32)
    pruned_att = np.take_along_axis(vals, idx, axis=1).reshape(-1)
    orig_indices = (np.arange(B, dtype=np.int32)[:, None] * EPG + idx).reshape(-1)
    pruned_edges = edges1[orig_indices]
    sm_pruned = sm1[orig_indices]

    # ---- node score aggregation ----
    updated_node_score = np.zeros(N_NODES, np.float32)
    tgt_p = np.clip(pruned_edges[:, 7], 0, N_NODES - 1).astype(np.int64)
    np.add.at(updated_node_score, tgt_p, sm_pruned * pruned_att)

    # ---- propagate representations along pruned edges ----
    src_p = np.clip(pruned_edges[:, 6], 0, N_NODES - 1).astype(np.int64)
    agg = np.zeros_like(mem)
    np.add.at(agg, tgt_p, sm_pruned[:, None] * mem[src_p])
    cnt = np.zeros(N_NODES, np.float32)
    np.add.at(cnt, tgt_p, 1.0)
    emb1 = np.where((cnt > 0)[:, None], agg, mem)

    # ---- layer 0 ----
    seg0 = np.clip(edges0[:, 6], 0, N_NODES - 1).astype(np.int64)
    sm0 = _sm_for_layer(edges0, emb1, rel_emb0, qs_tab, qr_tab, W, trace)
    tgt0 = np.clip(edges0[:, 7], 0, N_NODES - 1).astype(np.int64)
    agg0 = np.zeros_like(emb1)
    np.add.at(agg0, tgt0, sm0[:, None] * emb1[seg0])
    cnt0 = np.zeros(N_NODES, np.float32)
    np.add.at(cnt0, tgt0, 1.0)
    emb2 = np.where((cnt0 > 0)[:, None], agg0, emb1)

    # ---- bypass linear + LeakyReLU on device ----
    emb_out = _device_linear(emb2, np.asarray(Wlin, np.float32),
                             np.asarray(blin, np.float32), trace)

    return (updated_node_score, emb_out,
            pruned_edges.astype(np.int32), orig_indices.astype(np.int32))


# revision 6
# speedup vs baseline: 3.2315x; 1.0638x over previous
"""Trainium2 Bass kernel for nn_AttentionFlow (gnn_message_passing).

Strategy: the dominant compute — the per-edge bilinear MLP attention scores
for both GNN layers (2 x ~103 GFLOP) and the final 100000x256x256 linear
(13 GFLOP) — runs on the 8 NeuronCores in fp32 via Bass/Tile kernels,
sharded by edges (8192 edges/core, matching the eg-group sharding hint) and
by node ranges for the final linear.  Host code performs index-derived
staging (gathers into transposed feature blocks, segment bookkeeping,
top-k assembly) and the cross-shard unshard/merge.
"""
import os
import sys

sys.path.insert(0, '/opt/trn_rl_repo')

import numpy as np

N_NODES = 100000
D = 256
B = 64
EPG = 1024
E = 65536
NCORES = 8
ESH = 3072                 # multi-edge capacity per core (6 tiles of 512)
TN = 512                   # moving-dim tile (edges per matmul)
NT = ESH // TN             # 6 tiles per core
KFULL = 2 * D + 128        # 640 contraction rows: [hi|rel|onehot(eg)+pad]
DH = 2 * D                 # 512 hidden features
NROWS_LIN = 12800          # padded node rows per core for the final linear
NT_LIN = NROWS_LIN // TN   # 25

_cache = {}
LAST_EXEC_NS = []


def _enable_tracing_if_requested():
    if not os.environ.get("KERNEL_TRACE"):
        return False
    try:
        import types
        import antenv
        if 'antenv.axon_hooks' not in sys.modules:
            mod = types.ModuleType('antenv.axon_hooks')
            _store = [None]
            mod.set_axon_ntff_profile_hook = lambda h: _store.__setitem__(0, h)
            mod.get_axon_ntff_profile_hook = lambda: _store[0]
            sys.modules['antenv.axon_hooks'] = mod
            antenv.axon_hooks = mod
            from trn_agent_boot.trn_boot import _ntff_profile_via_ctypes
            mod.set_axon_ntff_profile_hook(
                _ntff_profile_via_ctypes('/opt/axon/libaxon_pjrt.so'))
            from concourse import bass_utils
            bass_utils.upload_artifacts = lambda tmpdir: f"local:{tmpdir}"
        return True
    except Exception:
        return False


def _build_scorer():
    """Per-core edge MLP scorer.

    in:  lT [1024, 8192]  (left features, transposed:  [hi; rel; qs; qr].T)
         rT [1024, 8192]  (right features, transposed: [hj; rel; qs; qr].T)
         Wl, Wr [1024, 512], Wc [512, 512], bl, br, bc [512]
    out: logits [1, 8192]
         logits[e] = sum(LReLU(l @ Wl + bl) * (LReLU(r @ Wr + br) @ Wc + bc))
    """
    import concourse.mybir as mybir
    import concourse.tile as tile
    from concourse import bacc

    f32 = mybir.dt.float32
    nc = bacc.Bacc("TRN2", target_bir_lowering=False, debug=False,
                   num_devices=NCORES)
    lT = nc.dram_tensor("lT", [KFULL, ESH], f32, kind="ExternalInput").ap()
    rT = nc.dram_tensor("rT", [KFULL, ESH], f32, kind="ExternalInput").ap()
    Wl = nc.dram_tensor("Wl", [KFULL, DH], f32, kind="ExternalInput").ap()
    Wr = nc.dram_tensor("Wr", [KFULL, DH], f32, kind="ExternalInput").ap()
    Wc = nc.dram_tensor("Wc", [DH, DH], f32, kind="ExternalInput").ap()
    bl = nc.dram_tensor("bl", [DH], f32, kind="ExternalInput").ap()
    br = nc.dram_tensor("br", [DH], f32, kind="ExternalInput").ap()
    bc = nc.dram_tensor("bc", [DH], f32, kind="ExternalInput").ap()
    logits = nc.dram_tensor("logits", [1, ESH], f32, kind="ExternalOutput").ap()

    KC = KFULL // 128   # 8 contraction chunks for Wl/Wr
    MC = DH // 128      # 4 output-feature chunks
    KC2 = DH // 128     # 4 contraction chunks for Wc

    with tile.TileContext(nc) as tc:
        with tc.tile_pool(name="wpool", bufs=1) as wp, \
             tc.tile_pool(name="stream", bufs=3) as sp, \
             tc.tile_pool(name="work", bufs=2) as kp, \
             tc.tile_pool(name="psum", bufs=2, space="PSUM") as pp, \
             tc.tile_pool(name="psd", bufs=2, space="PSUM") as pd:
            # --- preload weights / biases (resident) ---
            wl_t = [wp.tile([128, DH], f32, name=f"wl{k}", tag=f"wl{k}") for k in range(KC)]
            wr_t = [wp.tile([128, DH], f32, name=f"wr{k}", tag=f"wr{k}") for k in range(KC)]
            wc_t = [wp.tile([128, DH], f32, name=f"wc{k}", tag=f"wc{k}") for k in range(KC2)]
            for k in range(KC):
                nc.sync.dma_start(out=wl_t[k][:], in_=Wl[k * 128:(k + 1) * 128, :])
                nc.sync.dma_start(out=wr_t[k][:], in_=Wr[k * 128:(k + 1) * 128, :])
            for k in range(KC2):
                nc.sync.dma_start(out=wc_t[k][:], in_=Wc[k * 128:(k + 1) * 128, :])
            bl_t = wp.tile([128, MC], f32, tag="bl")
            br_t = wp.tile([128, MC], f32, tag="br")
            bc_t = wp.tile([128, MC], f32, tag="bc")
            nc.sync.dma_start(out=bl_t[:], in_=bl.rearrange("(c p) -> p c", p=128))
            nc.sync.dma_start(out=br_t[:], in_=br.rearrange("(c p) -> p c", p=128))
            nc.sync.dma_start(out=bc_t[:], in_=bc.rearrange("(c p) -> p c", p=128))
            ones_t = wp.tile([128, 1], f32, tag="ones")
            nc.vector.memset(ones_t[:], 1.0)

            for nt in range(NT):
                esl = slice(nt * TN, (nt + 1) * TN)
                # stream in the transposed feature chunks for this edge tile
                lch = []
                rch = []
                for k in range(KC):
                    t = sp.tile([128, TN], f32, name=f"lt{k}", tag=f"lt{k}")
                    nc.sync.dma_start(out=t[:], in_=lT[k * 128:(k + 1) * 128, esl])
                    lch.append(t)
                for k in range(KC):
                    t = sp.tile([128, TN], f32, name=f"rt{k}", tag=f"rt{k}")
                    nc.sync.dma_start(out=t[:], in_=rT[k * 128:(k + 1) * 128, esl])
                    rch.append(t)
                # l = LReLU(left @ Wl + bl)   (feature-major [DH, TN])
                l_sb = []
                for mc in range(MC):
                    ps = pp.tile([128, TN], f32, tag="ps", space="PSUM")
                    for k in range(KC):
                        nc.tensor.matmul(
                            ps[:], lhsT=wl_t[k][:, mc * 128:(mc + 1) * 128],
                            rhs=lch[k][:], start=(k == 0), stop=(k == KC - 1))
                    t = kp.tile([128, TN], f32, name=f"lsb{mc}", tag=f"lsb{mc}")
                    nc.scalar.activation(t[:], ps[:],
                                         mybir.ActivationFunctionType.Lrelu,
                                         bias=bl_t[:, mc:mc + 1], alpha=0.01)
                    l_sb.append(t)
                # rp = LReLU(right @ Wr + br)
                rp_sb = []
                for mc in range(MC):
                    ps = pp.tile([128, TN], f32, tag="ps", space="PSUM")
                    for k in range(KC):
                        nc.tensor.matmul(
                            ps[:], lhsT=wr_t[k][:, mc * 128:(mc + 1) * 128],
                            rhs=rch[k][:], start=(k == 0), stop=(k == KC - 1))
                    t = kp.tile([128, TN], f32, name=f"rpsb{mc}", tag=f"rpsb{mc}")
                    nc.scalar.activation(t[:], ps[:],
                                         mybir.ActivationFunctionType.Lrelu,
                                         bias=br_t[:, mc:mc + 1], alpha=0.01)
                    rp_sb.append(t)
                # r2 = rp @ Wc + bc ; acc = sum_mc l*r2 ; logits = colsum(acc)
                dps = pd.tile([1, TN], f32, tag="dot", space="PSUM")
                acc = kp.tile([128, TN], f32, tag="acc")
                for mc in range(MC):
                    ps = pp.tile([128, TN], f32, tag="ps", space="PSUM")
                    for k in range(KC2):
                        nc.tensor.matmul(
                            ps[:], lhsT=wc_t[k][:, mc * 128:(mc + 1) * 128],
                            rhs=rp_sb[k][:], start=(k == 0), stop=(k == KC2 - 1))
                    r2 = kp.tile([128, TN], f32, tag="r2")
                    nc.scalar.activation(r2[:], ps[:],
                                         mybir.ActivationFunctionType.Identity,
                                         bias=bc_t[:, mc:mc + 1])
                    if mc == 0:
                        nc.vector.tensor_mul(acc[:], l_sb[mc][:], r2[:])
                    else:
                        prod = kp.tile([128, TN], f32, tag="prod")
                        nc.vector.tensor_mul(prod[:], l_sb[mc][:], r2[:])
                        nc.vector.tensor_add(acc[:], acc[:], prod[:])
                nc.tensor.matmul(dps[:], lhsT=ones_t[:], rhs=acc[:],
                                 start=True, stop=True)
                lo = kp.tile([1, TN], f32, tag="lo")
                nc.vector.tensor_copy(lo[:], dps[:])
                nc.sync.dma_start(out=logits[:, esl], in_=lo[:])
    nc.compile()
    return nc


def _build_linear():
    """Per-core final linear: outT = LReLU(Wlin.T @ embT + blin).

    in:  embT [256, 12800], Wlin [256, 256], blin [256]
    out: outT [256, 12800]
    """
    import concourse.mybir as mybir
    import concourse.tile as tile
    from concourse import bacc

    f32 = mybir.dt.float32
    nc = bacc.Bacc("TRN2", target_bir_lowering=False, debug=False,
                   num_devices=NCORES)
    embT = nc.dram_tensor("embT", [D, NROWS_LIN], f32, kind="ExternalInput").ap()
    Wlin = nc.dram_tensor("Wlin", [D, D], f32, kind="ExternalInput").ap()
    blin = nc.dram_tensor("blin", [D], f32, kind="ExternalInput").ap()
    outT = nc.dram_tensor("outT", [D, NROWS_LIN], f32, kind="ExternalOutput").ap()

    KC = D // 128  # 2
    MC = D // 128  # 2
    with tile.TileContext(nc) as tc:
        with tc.tile_pool(name="wpool", bufs=1) as wp, \
             tc.tile_pool(name="stream", bufs=3) as sp, \
             tc.tile_pool(name="psum", bufs=2, space="PSUM") as pp:
            w_t = [wp.tile([128, D], f32, name=f"w{k}", tag=f"w{k}") for k in range(KC)]
            for k in range(KC):
                nc.sync.dma_start(out=w_t[k][:], in_=Wlin[k * 128:(k + 1) * 128, :])
            b_t = wp.tile([128, MC], f32, tag="b")
            nc.sync.dma_start(out=b_t[:], in_=blin.rearrange("(c p) -> p c", p=128))
            for nt in range(NT_LIN):
                esl = slice(nt * TN, (nt + 1) * TN)
                ech = []
                for k in range(KC):
                    t = sp.tile([128, TN], f32, name=f"et{k}", tag=f"et{k}")
                    nc.sync.dma_start(out=t[:], in_=embT[k * 128:(k + 1) * 128, esl])
                    ech.append(t)
                for mc in range(MC):
                    ps = pp.tile([128, TN], f32, tag="ps", space="PSUM")
                    for k in range(KC):
                        nc.tensor.matmul(
                            ps[:], lhsT=w_t[k][:, mc * 128:(mc + 1) * 128],
                            rhs=ech[k][:], start=(k == 0), stop=(k == KC - 1))
                    o = sp.tile([128, TN], f32, tag="o")
                    nc.scalar.activation(o[:], ps[:],
                                         mybir.ActivationFunctionType.Lrelu,
                                         bias=b_t[:, mc:mc + 1], alpha=0.01)
                    nc.sync.dma_start(out=outT[mc * 128:(mc + 1) * 128, esl],
                                      in_=o[:])
    nc.compile()
    return nc


def _run(nc, in_maps, trace):
    from concourse.bass_utils import run_bass_kernel_spmd
    res = run_bass_kernel_spmd(nc, in_maps, list(range(NCORES)), trace=trace)
    if trace:
        ns = None
        try:
            import glob
            import json
            f = max(glob.glob('/tmp/tmp*/ntff_0.json'), key=os.path.getmtime)
            ins = json.load(open(f))['instruction']
            ns = (max(r['timestamp'] + r['duration'] for r in ins)
                  - min(r['timestamp'] for r in ins))
        except Exception:
            ns = res.exec_time_ns
        if ns:
            LAST_EXEC_NS.append(ns)
    return res.results


def _device_logits_subset(edges, midx, emb, rel, qs_tab, qr_tab, W, trace):
    """Edge-MLP scores for the edge subset midx, on 8 cores (padded shards)."""
    Wl, bl, Wr, br, Wc, bc = W
    e = edges[midx]
    src = np.clip(e[:, 6], 0, N_NODES - 1).astype(np.int64)
    dst = np.clip(e[:, 7], 0, N_NODES - 1).astype(np.int64)
    eg = np.clip(e[:, 0], 0, B - 1).astype(np.int64)
    nM = len(midx)
    out = np.empty(nM, np.float32)
    common = {"Wl": np.ascontiguousarray(Wl), "Wr": np.ascontiguousarray(Wr),
              "Wc": np.ascontiguousarray(Wc), "bl": bl, "br": br, "bc": bc}
    CAP = NCORES * ESH
    for base in range(0, nM, CAP):
        hi_b = slice(base, min(base + CAP, nM))
        n_b = hi_b.stop - hi_b.start
        lT = np.zeros((KFULL, CAP), np.float32)
        rT = np.zeros((KFULL, CAP), np.float32)
        lT[:D, :n_b] = emb[src[hi_b]].T
        rT[:D, :n_b] = emb[dst[hi_b]].T
        lT[D:2 * D, :n_b] = rel[midx[hi_b]].T
        rT[D:2 * D, :n_b] = lT[D:2 * D, :n_b]
        lT[2 * D + eg[hi_b], np.arange(n_b)] = 1.0
        rT[2 * D:, :n_b] = lT[2 * D:, :n_b]
        in_maps = []
        for c in range(NCORES):
            sl = slice(c * ESH, (c + 1) * ESH)
            in_maps.append({"lT": np.ascontiguousarray(lT[:, sl]),
                            "rT": np.ascontiguousarray(rT[:, sl]), **common})
        results = _run(_cache["scorer"], in_maps, trace)
        lo = np.concatenate([results[c]["logits"][0] for c in range(NCORES)])
        out[hi_b] = lo[:n_b]
    return out


def _device_linear(emb2, Wlin, blin, trace):
    """LReLU(emb2 @ Wlin + blin) on 8 cores (node-range sharded)."""
    embT = np.zeros((D, NCORES * NROWS_LIN), dtype=np.float32)
    embT[:, :N_NODES] = emb2.T
    common = {"Wlin": np.ascontiguousarray(Wlin), "blin": blin}
    in_maps = []
    for c in range(NCORES):
        sl = slice(c * NROWS_LIN, (c + 1) * NROWS_LIN)
        in_maps.append({"embT": np.ascontiguousarray(embT[:, sl]), **common})
    results = _run(_cache["linear"], in_maps, trace)
    outT = np.concatenate([results[c]["outT"] for c in range(NCORES)], axis=1)
    return np.ascontiguousarray(outT[:, :N_NODES].T)


def _sm_for_layer(edges, emb, rel, qs_tab, qr_tab, W, trace):
    """Segment softmax over edges[:,6]; sm == 1.0 exactly for edges whose
    source node has a single edge, so only multi-edge-node edges are scored
    on device (the reference's e/s is exactly 1.0 there too)."""
    seg = np.clip(edges[:, 6], 0, N_NODES - 1).astype(np.int64)
    cnt = np.bincount(seg, minlength=N_NODES)
    multi = cnt[seg] >= 2
    midx = np.nonzero(multi)[0]
    sm = np.ones(len(edges), np.float32)
    if len(midx):
        lo = _device_logits_subset(edges, midx, emb, rel, qs_tab, qr_tab, W,
                                   trace)
        segm = seg[midx]
        m = np.full(N_NODES, -np.inf, np.float32)
        np.maximum.at(m, segm, lo)
        e = np.exp(lo - m[segm])
        s = np.zeros(N_NODES, np.float32)
        np.add.at(s, segm, e)
        sm[midx] = e / s[segm]
    return sm


def kernel(attended_nodes, node_score, edges0, edges1, rel_emb0, rel_emb1,
           memorized_embedding, query_src_ts_emb, query_rel_emb,
           Wl, bl, Wr, br, Wc, bc, Wlin, blin, max_edges):
    trace = _enable_tracing_if_requested()
    LAST_EXEC_NS.clear()
    if "scorer" not in _cache:
        _cache["scorer"] = _build_scorer()
    if "linear" not in _cache:
        _cache["linear"] = _build_linear()

    node_score = np.asarray(node_score, dtype=np.float32)
    edges0 = np.asarray(edges0, dtype=np.int32)
    edges1 = np.asarray(edges1, dtype=np.int32)
    rel_emb0 = np.asarray(rel_emb0, dtype=np.float32)
    rel_emb1 = np.asarray(rel_emb1, dtype=np.float32)
    mem = np.asarray(memorized_embedding, dtype=np.float32)
    qs_tab = np.asarray(query_src_ts_emb, dtype=np.float32)
    qr_tab = np.asarray(query_rel_emb, dtype=np.float32)
    Wl = np.asarray(Wl, np.float32)
    Wr = np.asarray(Wr, np.float32)
    q_cat = np.concatenate([qs_tab, qr_tab], axis=1).astype(np.float64)
    Wl_f = np.zeros((KFULL, DH), np.float32)
    Wl_f[:2 * D] = Wl[:2 * D]
    Wl_f[2 * D:2 * D + B] = (q_cat @ Wl[2 * D:].astype(np.float64)).astype(np.float32)
    Wr_f = np.zeros((KFULL, DH), np.float32)
    Wr_f[:2 * D] = Wr[:2 * D]
    Wr_f[2 * D:2 * D + B] = (q_cat @ Wr[2 * D:].astype(np.float64)).astype(np.float32)
    W = (Wl_f, np.asarray(bl, np.float32),
         Wr_f, np.asarray(br, np.float32),
         np.asarray(Wc, np.float32), np.asarray(bc, np.float32))
    K = int(max_edges)

    # ---- layer 1: edge MLP scores on device (multi-edge nodes only) ----
    seg1 = np.clip(edges1[:, 6], 0, N_NODES - 1).astype(np.int64)
    sm1 = _sm_for_layer(edges1, mem, rel_emb1, qs_tab, qr_tab, W, trace)
    target_att = sm1 * node_score[seg1]

    # ---- per-query top-k (stable: ties -> lower index, matching lax.top_k) --
    vals = target_att.reshape(B, EPG)
    idx = np.argsort(-vals, axis=1, kind="stable")[:, :K].astype(np.int32)
    pruned_att = np.take_along_axis(vals, idx, axis=1).reshape(-1)
    orig_indices = (np.arange(B, dtype=np.int32)[:, None] * EPG + idx).reshape(-1)
    pruned_edges = edges1[orig_indices]
    sm_pruned = sm1[orig_indices]

    # ---- node score aggregation ----
    updated_node_score = np.zeros(N_NODES, np.float32)
    tgt_p = np.clip(pruned_edges[:, 7], 0, N_NODES - 1).astype(np.int64)
    np.add.at(updated_node_score, tgt_p, sm_pruned * pruned_att)

    # ---- propagate representations along pruned edges ----
    src_p = np.clip(pruned_edges[:, 6], 0, N_NODES - 1).astype(np.int64)
    agg = np.zeros_like(mem)
    np.add.at(agg, tgt_p, sm_pruned[:, None] * mem[src_p])
    cnt = np.zeros(N_NODES, np.float32)
    np.add.at(cnt, tgt_p, 1.0)
    emb1 = np.where((cnt > 0)[:, None], agg, mem)

    # ---- layer 0 ----
    seg0 = np.clip(edges0[:, 6], 0, N_NODES - 1).astype(np.int64)
    sm0 = _sm_for_layer(edges0, emb1, rel_emb0, qs_tab, qr_tab, W, trace)
    tgt0 = np.clip(edges0[:, 7], 0, N_NODES - 1).astype(np.int64)
    agg0 = np.zeros_like(emb1)
    np.add.at(agg0, tgt0, sm0[:, None] * emb1[seg0])
    cnt0 = np.zeros(N_NODES, np.float32)
    np.add.at(cnt0, tgt0, 1.0)
    emb2 = np.where((cnt0 > 0)[:, None], agg0, emb1)

    # ---- bypass linear + LeakyReLU on device ----
    emb_out = _device_linear(emb2, np.asarray(Wlin, np.float32),
                             np.asarray(blin, np.float32), trace)

    return (updated_node_score, emb_out,
            pruned_edges.astype(np.int32), orig_indices.astype(np.int32))
